# revision 14
# baseline (speedup 1.0000x reference)
"""3-layer GCN (DGL GraphConv, norm='both') on 8 Trainium2 NeuronCores.

v4: descriptor-minimized, software-pipelined SPMD single-NEFF design.
  - Nodes partitioned contiguously: core c owns rows [c*12500, (c+1)*12500).
  - Per layer: project own nodes on PE (bf16) -> [12500,128] bf16 shard;
    AllGather split into FOUR quarter-collectives that fire as soon as their
    projection windows complete (table chunk q = concat over cores of local
    rows [q*3125,(q+1)*3125), keeping int16 gather indices valid).
  - Per-edge SWDGE dma_gather fetches source rows; calls round-robin over 4
    SWDGE queues so all four Q7 core pairs generate descriptors in parallel.
  - Gather stream layout: cells (dst-window, src-chunk) sized to the
    cross-core max count and concatenated per (super-group, chunk) WITHOUT
    per-cell rounding (only the group tail pads to a tile). Tiles may span
    window boundaries; each (window, chunk, tile) matmul gets its own
    one-hot column block with -1 (no match) marking rows of other windows.
    This cuts gathered rows ~12% vs per-cell 128-rounding.
  - Segment-sum by dst via one-hot matmul accumulation in PSUM over 128-dst
    windows; one-hot built in wide DVE is_equal instrs.
  - Next-layer projection windows are emitted right after each super-group's
    aggregation, keeping PE/DMA/Q7 busy across layer boundaries.
  - Layers 1-2 keep h transposed ([feat x nodes]); the in-degree norm is
    deferred into the next projection's per-row scale (zero bias asserted).
  - Host (numpy) does index-only prep (degrees, bucketing, sorting, padding
    to a core-uniform static schedule - SPMD needs identical instruction
    streams on all 8 cores).
"""

import numpy as np
import ml_dtypes

import concourse.bacc as bacc
import concourse.bass as bass
import concourse.mybir as mybir
import concourse.tile as tile
from concourse.bass import AP
from concourse.bass_utils import run_bass_kernel_spmd

BF16 = ml_dtypes.bfloat16
F32 = np.float32

N_NODES = 100000
D_IN, D_H1, D_H2, D_OUT = 256, 128, 128, 64
NCORE = 8
NP = N_NODES // NCORE        # 12500 nodes per core
NWIN = (NP + 127) // 128     # 98 windows (last holds 84)
NPPAD = NWIN * 128           # 12544
NCHUNK = 4
QROWS = NP // NCHUNK         # 3125 rows per quarter-AG per core
CHUNK = QROWS * NCORE        # 25000 rows per gather chunk
GCALL = 1024                 # idxs per dma_gather call
SUPW = 4                     # windows per gather super-group
NQUEUE = 4                   # SWDGE queues (Q7 core pairs)
FTW = 8                      # windows per staged featT slice (L0)
# last proj window needed by quarter-AG q
AGWIN = [((QROWS * (q + 1) + 127) // 128) - 1 for q in range(NCHUNK)]


def _host_prep(feat, W1, b1, W2, b2, W3, b3, src, dst):
    src = np.asarray(src).astype(np.int64)
    dst = np.asarray(dst).astype(np.int64)
    for b in (b1, b2, b3):
        assert np.max(np.abs(np.asarray(b))) == 0.0, \
            "nonzero bias needs the undeferred-nd path"

    deg_out = np.bincount(src, minlength=N_NODES).astype(F32)
    deg_in = np.bincount(dst, minlength=N_NODES).astype(F32)
    ns = 1.0 / np.sqrt(np.maximum(deg_out, 1.0))
    nd = 1.0 / np.sqrt(np.maximum(deg_in, 1.0))
    nsd = ns * nd

    core = dst // NP
    dloc = dst % NP
    win = dloc // 128
    c_src = src // NP
    r_src = src % NP
    chunk = r_src // QROWS
    sloc = (c_src * QROWS + (r_src % QROWS)).astype(np.int16)
    dcol = (dloc % 128).astype(np.int32)

    ncell = NCORE * NWIN * NCHUNK
    cell = ((core * NWIN + win) * NCHUNK + chunk).astype(np.int64)
    order = np.argsort(cell * CHUNK + sloc, kind="stable")
    counts = np.bincount(cell, minlength=ncell).reshape(NCORE, NWIN, NCHUNK)
    m_wj = counts.max(axis=0).astype(np.int64)      # uniform cell sizes

    starts = np.zeros(ncell + 1, np.int64)
    np.cumsum(np.bincount(cell, minlength=ncell), out=starts[1:])

    nsup = (NWIN + SUPW - 1) // SUPW
    sup_ws = [list(range(s * SUPW, min((s + 1) * SUPW, NWIN)))
              for s in range(nsup)]

    # slot layout: per (super, chunk) group: cells concatenated (no per-cell
    # rounding), group padded to a tile boundary.
    A_wj = np.zeros((NWIN, NCHUNK), np.int64)   # absolute slot offset of cell
    gtile0 = {}                                  # (s, j) -> first abs tile
    gtiles = {}                                  # (s, j) -> tiles in group
    sup_tile0 = []                               # first abs tile of super
    p = 0                                        # in slots (always 128-aligned
    #                                              at group boundaries)
    for s in range(nsup):
        sup_tile0.append(p // 128)
        for j in range(NCHUNK):
            gtile0[(s, j)] = p // 128
            g0 = p
            for w in sup_ws[s]:
                A_wj[w, j] = p
                p += int(m_wj[w, j])
            p = ((p + 127) // 128) * 128
            gtiles[(s, j)] = (p - g0) // 128
    NT2 = p // 128                               # total stream tiles
    sup_tile0.append(NT2)

    # matmul list per super: (w, j, abs_tile) in emission order; per-window
    # start/stop counts
    mm_of_sup = []                               # list per super of (w,j,t)
    nmm_w = np.zeros(NWIN, np.int64)
    for s in range(nsup):
        lst = []
        for w in sup_ws[s]:
            for j in range(NCHUNK):
                if m_wj[w, j] == 0:
                    continue
                a, m = int(A_wj[w, j]), int(m_wj[w, j])
                for t in range(a // 128, (a + m + 127) // 128):
                    lst.append((w, j, t))
                    nmm_w[w] += 1
        mm_of_sup.append(lst)
    NMM = int(sum(len(x) for x in mm_of_sup))
    max_sup_mm = max(len(x) for x in mm_of_sup)
    max_sup_tiles = max(sup_tile0[s + 1] - sup_tile0[s] for s in range(nsup))

    # per-core slot data
    sidx = np.full((NCORE, NT2 * 128), -1, np.int16)
    dcol_slot = np.full((NCORE, NT2 * 128), -1, np.int64)
    for c in range(NCORE):
        for w in range(NWIN):
            for j in range(NCHUNK):
                cid = (c * NWIN + w) * NCHUNK + j
                e = order[starts[cid]:starts[cid + 1]]
                n = len(e)
                a = int(A_wj[w, j])
                sidx[c, a:a + n] = sloc[e]
                sidx[c, a + n:a + int(m_wj[w, j])] = 0   # interior pad
                dcol_slot[c, a:a + n] = dcol[e]
    # group tails keep idx=-1 (stripped at call end by the Q7)

    # per-matmul one-hot columns: [NMM, 128] values or -1
    dcol_mm = np.full((NCORE, NMM, 128), -1.0, F32)
    mm_idx = 0
    for s in range(nsup):
        for (w, j, t) in mm_of_sup[s]:
            a, m = int(A_wj[w, j]), int(m_wj[w, j])
            lo = max(a, t * 128)
            hi = min(a + m, (t + 1) * 128)
            for c in range(NCORE):
                seg = dcol_slot[c, lo:hi]
                dst_rows = np.arange(lo - t * 128, hi - t * 128)
                valid = seg >= 0
                dcol_mm[c, mm_idx, dst_rows[valid]] = seg[valid]
            mm_idx += 1
    assert mm_idx == NMM

    def idx_layout(a):      # [n] int16 -> [128, n//16]
        return np.tile(a.reshape(-1, 16).T, (8, 1))

    sidx_l = np.stack([idx_layout(sidx[c]) for c in range(NCORE)])
    dcol_l = np.ascontiguousarray(
        dcol_mm.transpose(0, 2, 1)).astype(BF16)    # [NCORE, 128, NMM]

    calls = []   # (chunk j, abs slot offset, n_idxs, super)
    for s in range(nsup):
        for j in range(NCHUNK):
            t0, nt = gtile0[(s, j)], gtiles[(s, j)]
            off = t0 * 128
            greal = int(sum(m_wj[w, j] for w in sup_ws[s]))   # real slots
            q = 0
            while q < greal:
                n = min(GCALL, ((greal - q + 15) // 16) * 16)
                calls.append((j, off + q, n, s))
                q += n

    feat = np.asarray(feat).astype(F32)
    featT = np.zeros((NCORE, 2, 128, NPPAD), BF16)
    nsp = np.zeros((NCORE, 128, NWIN), F32)
    nsdp = np.zeros((NCORE, 128, NWIN), F32)
    ndp = np.zeros((NCORE, 128, NWIN), F32)
    for c in range(NCORE):
        ft = feat[c * NP:(c + 1) * NP].astype(BF16).T   # [256, NP]
        featT[c, 0, :, :NP] = ft[0:128]
        featT[c, 1, :, :NP] = ft[128:256]
        for arr, dstp in ((ns, nsp), (nsd, nsdp), (nd, ndp)):
            v = np.zeros(NPPAD, F32)
            v[:NP] = arr[c * NP:(c + 1) * NP]
            dstp[c] = v.reshape(NWIN, 128).T

    ohchunk = min(32, max_sup_mm)
    consts = dict(
        w1=np.asarray(W1).astype(F32).astype(BF16),
        w2=np.asarray(W2).astype(F32).astype(BF16),
        w3p=np.pad(np.asarray(W3).astype(F32), ((0, 0), (0, 128 - D_OUT))).astype(BF16),
        iota=np.tile(np.arange(128, dtype=F32).astype(BF16)[None, :],
                     (128, ohchunk)),
    )
    sched = dict(NT2=NT2, NMM=NMM, calls=calls, mm_of_sup=mm_of_sup,
                 nmm_w=nmm_w, sup_tile0=sup_tile0, nsup=nsup, sup_ws=sup_ws,
                 max_sup_tiles=max_sup_tiles, max_sup_mm=max_sup_mm,
                 ohchunk=ohchunk)
    percore = dict(featT=featT, nsp=nsp, nsdp=nsdp, ndp=ndp,
                   sidx=sidx_l, dcol=dcol_l)
    return sched, consts, percore


def _build(sched):
    NT2 = sched["NT2"]; NMM = sched["NMM"]; calls = sched["calls"]
    mm_of_sup = sched["mm_of_sup"]; nmm_w = sched["nmm_w"]
    sup_tile0 = sched["sup_tile0"]; nsup = sched["nsup"]
    sup_ws = sched["sup_ws"]; max_sup_tiles = sched["max_sup_tiles"]
    max_sup_mm = sched["max_sup_mm"]; ohchunk = sched["ohchunk"]

    calls_by_sup = {}
    for c in calls:
        calls_by_sup.setdefault(c[3], []).append(c)
    # first matmul index (within super) per super, per window: for start/stop
    nc = bacc.Bacc("TRN2", target_bir_lowering=False, debug=False,
                   num_devices=NCORE, num_swdge_queues=NQUEUE)
    dt = mybir.dt

    featT_t = nc.dram_tensor("featT", [2, 128, NPPAD], dt.bfloat16,
                             kind="ExternalInput")
    w1_t = nc.dram_tensor("w1", [D_IN, D_H1], dt.bfloat16, kind="ExternalInput")
    w2_t = nc.dram_tensor("w2", [D_H1, D_H2], dt.bfloat16, kind="ExternalInput")
    w3_t = nc.dram_tensor("w3p", [D_H2, 128], dt.bfloat16, kind="ExternalInput")
    ns_t = nc.dram_tensor("nsp", [128, NWIN], dt.float32, kind="ExternalInput")
    nsd_t = nc.dram_tensor("nsdp", [128, NWIN], dt.float32, kind="ExternalInput")
    nd_t = nc.dram_tensor("ndp", [128, NWIN], dt.float32, kind="ExternalInput")
    sidx_t = nc.dram_tensor("sidx", [128, NT2 * 8], dt.int16, kind="ExternalInput")
    dcol_t = nc.dram_tensor("dcol", [128, NMM], dt.bfloat16, kind="ExternalInput")
    iota_t = nc.dram_tensor("iota", [128, ohchunk * 128], dt.bfloat16,
                            kind="ExternalInput")
    out_t = nc.dram_tensor("out", [NP, D_OUT], dt.float32, kind="ExternalOutput")

    qcount = [0]

    def next_queue():
        q = qcount[0] % NQUEUE
        qcount[0] += 1
        return q

    with tile.TileContext(nc) as tc:
        with (
            tc.tile_pool(name="const", bufs=1) as cpool,
            tc.tile_pool(name="hbuf", bufs=1) as hpool,
            tc.tile_pool(name="gb", bufs=3) as gpool,
            tc.tile_pool(name="ft", bufs=2) as ftpool,
            tc.tile_pool(name="work", bufs=3) as wpool,
            tc.tile_pool(name="oh", bufs=2) as ohpool,
            tc.tile_pool(name="ps", bufs=5, space="PSUM") as ppool,
            tc.tile_pool(name="pj", bufs=2, space="PSUM") as pjpool,
            tc.tile_pool(name="dram", bufs=1, space="DRAM") as dpool,
        ):
            w1a_s = cpool.tile([128, D_H1], dt.bfloat16)
            w1b_s = cpool.tile([128, D_H1], dt.bfloat16)
            w2_s = cpool.tile([D_H1, D_H2], dt.bfloat16)
            w3_s = cpool.tile([D_H2, 128], dt.bfloat16)
            ns_s = cpool.tile([128, NWIN], dt.float32)
            nsd_s = cpool.tile([128, NWIN], dt.float32)
            nd_s = cpool.tile([128, NWIN], dt.float32)
            sidx_s = cpool.tile([128, NT2 * 8], dt.int16)
            dcol_s = cpool.tile([128, NMM], dt.bfloat16)
            iota_s = cpool.tile([128, ohchunk * 128], dt.bfloat16)

            nc.sync.dma_start(w1a_s[:], w1_t.ap()[0:128, :])
            nc.sync.dma_start(w1b_s[:], w1_t.ap()[128:256, :])
            nc.sync.dma_start(w2_s[:], w2_t.ap())
            nc.sync.dma_start(w3_s[:], w3_t.ap())
            nc.sync.dma_start(ns_s[:], ns_t.ap())
            nc.sync.dma_start(nsd_s[:], nsd_t.ap())
            nc.sync.dma_start(nd_s[:], nd_t.ap())
            nc.sync.dma_start(sidx_s[:], sidx_t.ap())
            nc.sync.dma_start(dcol_s[:], dcol_t.ap())
            nc.sync.dma_start(iota_s[:], iota_t.ap())

            h_s = hpool.tile([128, NWIN * 128], dt.bfloat16)   # hT (feat x nodes)

            tins = [dpool.tile([NP, 128], dt.bfloat16, name=f"tin{L}")
                    for L in range(3)]
            tfulls = [[dpool.tile([CHUNK, 128], dt.bfloat16,
                                  name=f"tfull{L}_{q}", addr_space="Shared")
                       for q in range(NCHUNK)] for L in range(3)]

            # zero the two gather buffers once: stale bytes multiply with
            # one-hot zeros, so they must be finite (not NaN bit patterns)
            for _ in range(3):
                gz = gpool.tile([128, max_sup_tiles, 128], dt.bfloat16,
                                name="gsz", tag="gs",
                                padded_shape=[128, max_sup_tiles, 128])
                nc.vector.memset(gz[:], 0.0)

            def proj4(L, w0, wn, fta=None, ftb=None, k0=0):
                # wn (<=4) projection windows into one PSUM bank, one wide
                # scale instr, per-window tin writes
                ppj = pjpool.tile([128, wn * 128], dt.float32, name=f"pj{L}",
                                  tag="pj", padded_shape=[128, 512])
                for k in range(wn):
                    w = w0 + k
                    osl = ppj[:, k * 128:(k + 1) * 128]
                    if L == 0:
                        nc.tensor.matmul(osl,
                                         lhsT=fta[:, (k0 + k) * 128:(k0 + k + 1) * 128],
                                         rhs=w1a_s[:], start=True, stop=False)
                        nc.tensor.matmul(osl,
                                         lhsT=ftb[:, (k0 + k) * 128:(k0 + k + 1) * 128],
                                         rhs=w1b_s[:], start=False, stop=True)
                    else:
                        rhs = w2_s if L == 1 else w3_s
                        nc.tensor.matmul(osl,
                                         lhsT=h_s[:, w * 128:(w + 1) * 128],
                                         rhs=rhs[:], start=True, stop=True)
                scal = ns_s if L == 0 else nsd_s
                ssl = scal[:, w0:w0 + wn]
                sbc = AP(ssl.tensor, ssl.offset, list(ssl.ap) + [[0, 128]])
                pbf = wpool.tile([128, wn * 128], dt.bfloat16, name="pbf",
                                 tag="pbf", padded_shape=[128, 512])
                nc.vector.tensor_tensor(out=pbf[:, 0:wn * 128],
                                        in0=ppj[:, 0:wn * 128], in1=sbc,
                                        op=mybir.AluOpType.mult)
                for k in range(wn):
                    w = w0 + k
                    wsz = min(128, NP - w * 128)
                    nc.sync.dma_start(tins[L][w * 128:w * 128 + wsz, :],
                                      pbf[:wsz, k * 128:(k + 1) * 128])

            def ag(L, q):
                nc.gpsimd.collective_compute(
                    "AllGather", mybir.AluOpType.bypass,
                    replica_groups=[list(range(NCORE))],
                    ins=[tins[L][q * QROWS:(q + 1) * QROWS, :].opt()],
                    outs=[tfulls[L][q][:].opt()],
                )

            def agg(L, s, ag_hooks=None):
                stile0 = sup_tile0[s]
                stiles = sup_tile0[s + 1] - stile0
                smms = mm_of_sup[s]
                nmm_s = len(smms)
                mm0 = sum(len(mm_of_sup[ss]) for ss in range(s))
                gs = gpool.tile([128, stiles, 128], dt.bfloat16,
                                name=f"gs{L}_{s}", tag="gs",
                                padded_shape=[128, max_sup_tiles, 128])
                for (j, off, n, cs) in calls_by_sup.get(s, []):
                    if ag_hooks and j in ag_hooks:
                        ag_hooks.pop(j)()
                    rel = off // 128 - stile0
                    nc.gpsimd.dma_gather(
                        gs[:, rel:rel + (n + 127) // 128, :],
                        tfulls[L][j][:],
                        sidx_s[:, off // 16:(off + n) // 16],
                        n, n, 128,
                        queue_num=next_queue(),
                    )
                if ag_hooks:
                    for j in sorted(ag_hooks):
                        ag_hooks[j]()
                oh = ohpool.tile([128, nmm_s * 128], dt.bfloat16,
                                 name=f"oh{L}", tag="oh",
                                 padded_shape=[128, max_sup_mm * 128])
                q = 0
                while q < nmm_s:
                    nb = min(ohchunk, nmm_s - q)
                    dsl = dcol_s[:, mm0 + q:mm0 + q + nb]
                    bcast = AP(dsl.tensor, dsl.offset,
                               list(dsl.ap) + [[0, 128]])
                    nc.vector.tensor_tensor(
                        out=oh[:, q * 128:(q + nb) * 128],
                        in0=iota_s[:, 0:nb * 128],
                        in1=bcast,
                        op=mybir.AluOpType.is_equal)
                    q += nb
                # per-window PSUM accumulation over this super's matmul list
                aps_of_w = {}
                done_of_w = {}
                for mi, (w, j, t) in enumerate(smms):
                    if w not in aps_of_w:
                        aps_of_w[w] = ppool.tile([128, 128], dt.float32,
                                                 name=f"ap{L}", tag="pp")
                        done_of_w[w] = 0
                    aps = aps_of_w[w]
                    k = done_of_w[w]
                    ohsl = oh[:, mi * 128:(mi + 1) * 128]
                    first, last = k == 0, k == int(nmm_w[w]) - 1
                    if L < 2:
                        nc.tensor.matmul(aps[:], lhsT=gs[:, t - stile0, :],
                                         rhs=ohsl, start=first, stop=last)
                    else:
                        nc.tensor.matmul(aps[:], lhsT=ohsl,
                                         rhs=gs[:, t - stile0, :],
                                         start=first, stop=last)
                    done_of_w[w] = k + 1
                    if not last:
                        continue
                    if L < 2:
                        nc.scalar.activation(
                            h_s[:, w * 128:(w + 1) * 128], aps[:],
                            mybir.ActivationFunctionType.Relu)
                    else:
                        wsz = min(128, NP - w * 128)
                        ob = wpool.tile([128, D_OUT], dt.float32, name="ob",
                                        tag="ob")
                        nc.vector.tensor_scalar(
                            out=ob[:], in0=aps[:, 0:D_OUT],
                            scalar1=nd_s[:, w:w + 1], scalar2=None,
                            op0=mybir.AluOpType.mult)
                        nc.sync.dma_start(
                            out_t.ap()[w * 128:w * 128 + wsz, :], ob[:wsz, :])

            # ---- L0 projection (staged featT slices) + quarter-AGs ----
            agq = 0
            for w0 in range(0, NWIN, FTW):
                wn = min(FTW, NWIN - w0)
                fta = ftpool.tile([128, wn * 128], dt.bfloat16, name="fta",
                                  tag="fta", padded_shape=[128, FTW * 128])
                ftb = ftpool.tile([128, wn * 128], dt.bfloat16, name="ftb",
                                  tag="ftb", padded_shape=[128, FTW * 128])
                nc.sync.dma_start(
                    fta[:], featT_t.ap()[0, :, w0 * 128:(w0 + wn) * 128])
                nc.sync.dma_start(
                    ftb[:], featT_t.ap()[1, :, w0 * 128:(w0 + wn) * 128])
                for g0 in range(0, wn, 4):
                    gn = min(4, wn - g0)
                    proj4(0, w0 + g0, gn, fta, ftb, g0)
                    while agq < 1 and w0 + g0 + gn - 1 >= AGWIN[agq]:
                        ag(0, agq)
                        agq += 1

            # ---- pipelined layers ----
            agsup = [min(nsup - 1, (AGWIN[q] // SUPW) + 2) for q in range(NCHUNK)]
            for L in range(3):
                if L == 0:
                    hooks = {q: (lambda qq=q: ag(0, qq)) for q in (1, 2, 3)}
                else:
                    hooks = {3: (lambda LL=L: ag(LL, 3))}
                nagq = 0
                for s in range(nsup):
                    agg(L, s, ag_hooks=hooks if s == 0 else None)
                    if L < 2:
                        proj4(L + 1, sup_ws[s][0], len(sup_ws[s]))
                        while nagq < NCHUNK - 1 and s >= agsup[nagq]:
                            ag(L + 1, nagq)
                            nagq += 1

    nc.compile()
    return nc


def _in_map(consts, percore, c):
    return {
        "featT": percore["featT"][c],
        "w1": consts["w1"], "w2": consts["w2"], "w3p": consts["w3p"],
        "nsp": percore["nsp"][c], "nsdp": percore["nsdp"][c],
        "ndp": percore["ndp"][c],
        "sidx": percore["sidx"][c], "dcol": percore["dcol"][c],
        "iota": consts["iota"],
    }


def kernel(feat, W1, b1, W2, b2, W3, b3, src, dst):
    sched, consts, percore = _host_prep(feat, W1, b1, W2, b2, W3, b3, src, dst)
    nc = _build(sched)
    in_maps = [_in_map(consts, percore, c) for c in range(NCORE)]
    res = run_bass_kernel_spmd(nc, in_maps, core_ids=list(range(NCORE)))
    out = np.concatenate([res.results[c]["out"][:NP] for c in range(NCORE)],
                         axis=0)
    return np.ascontiguousarray(out.astype(np.float32))


# revision 15
# speedup vs baseline: 1.1399x; 1.1399x over previous
"""3-layer GCN (DGL GraphConv, norm='both') on 8 Trainium2 NeuronCores.

v4: descriptor-minimized, software-pipelined SPMD single-NEFF design.
  - Nodes partitioned contiguously: core c owns rows [c*12500, (c+1)*12500).
  - Per layer: project own nodes on PE (bf16) -> [12500,128] bf16 shard;
    AllGather split into FOUR quarter-collectives that fire as soon as their
    projection windows complete (table chunk q = concat over cores of local
    rows [q*3125,(q+1)*3125), keeping int16 gather indices valid).
  - Per-edge SWDGE dma_gather fetches source rows; calls round-robin over 4
    SWDGE queues so all four Q7 core pairs generate descriptors in parallel.
  - Gather stream layout: cells (dst-window, src-chunk) sized to the
    cross-core max count and concatenated per (super-group, chunk) WITHOUT
    per-cell rounding (only the group tail pads to a tile). Tiles may span
    window boundaries; each (window, chunk, tile) matmul gets its own
    one-hot column block with -1 (no match) marking rows of other windows.
    This cuts gathered rows ~12% vs per-cell 128-rounding.
  - Segment-sum by dst via one-hot matmul accumulation in PSUM over 128-dst
    windows; one-hot built in wide DVE is_equal instrs.
  - Next-layer projection windows are emitted right after each super-group's
    aggregation, keeping PE/DMA/Q7 busy across layer boundaries.
  - Layers 1-2 keep h transposed ([feat x nodes]); the in-degree norm is
    deferred into the next projection's per-row scale (zero bias asserted).
  - Host (numpy) does index-only prep (degrees, bucketing, sorting, padding
    to a core-uniform static schedule - SPMD needs identical instruction
    streams on all 8 cores).
"""

import numpy as np
import ml_dtypes

import concourse.bacc as bacc
import concourse.bass as bass
import concourse.mybir as mybir
import concourse.tile as tile
from concourse.bass import AP
from concourse.bass_utils import run_bass_kernel_spmd

BF16 = ml_dtypes.bfloat16
F32 = np.float32

N_NODES = 100000
D_IN, D_H1, D_H2, D_OUT = 256, 128, 128, 64
NCORE = 8
NP = N_NODES // NCORE        # 12500 nodes per core
NWIN = (NP + 127) // 128     # 98 windows (last holds 84)
NPPAD = NWIN * 128           # 12544
NCHUNK = 4
QROWS = NP // NCHUNK         # 3125 rows per quarter-AG per core
CHUNK = QROWS * NCORE        # 25000 rows per gather chunk
GCALL = 1024                 # idxs per dma_gather call
SUPW = 4                     # windows per gather super-group
NQUEUE = 4                   # SWDGE queues (Q7 core pairs)
FTW = 8                      # windows per staged featT slice (L0)
# last proj window needed by quarter-AG q
AGWIN = [((QROWS * (q + 1) + 127) // 128) - 1 for q in range(NCHUNK)]


def _host_prep(feat, W1, b1, W2, b2, W3, b3, src, dst):
    src = np.asarray(src).astype(np.int64)
    dst = np.asarray(dst).astype(np.int64)
    for b in (b1, b2, b3):
        assert np.max(np.abs(np.asarray(b))) == 0.0, \
            "nonzero bias needs the undeferred-nd path"

    deg_out = np.bincount(src, minlength=N_NODES).astype(F32)
    deg_in = np.bincount(dst, minlength=N_NODES).astype(F32)
    ns = 1.0 / np.sqrt(np.maximum(deg_out, 1.0))
    nd = 1.0 / np.sqrt(np.maximum(deg_in, 1.0))
    nsd = ns * nd

    core = dst // NP
    dloc = dst % NP
    win = dloc // 128
    c_src = src // NP
    r_src = src % NP
    chunk = r_src // QROWS
    sloc = (c_src * QROWS + (r_src % QROWS)).astype(np.int16)
    dcol = (dloc % 128).astype(np.int32)

    ncell = NCORE * NWIN * NCHUNK
    cell = ((core * NWIN + win) * NCHUNK + chunk).astype(np.int64)
    order = np.argsort(cell * CHUNK + sloc, kind="stable")
    counts = np.bincount(cell, minlength=ncell).reshape(NCORE, NWIN, NCHUNK)
    m_wj = counts.max(axis=0).astype(np.int64)      # uniform cell sizes

    starts = np.zeros(ncell + 1, np.int64)
    np.cumsum(np.bincount(cell, minlength=ncell), out=starts[1:])

    nsup = (NWIN + SUPW - 1) // SUPW
    sup_ws = [list(range(s * SUPW, min((s + 1) * SUPW, NWIN)))
              for s in range(nsup)]

    # slot layout: per (super, chunk) group: cells concatenated (no per-cell
    # rounding), group padded to a tile boundary.
    A_wj = np.zeros((NWIN, NCHUNK), np.int64)   # absolute slot offset of cell
    gtile0 = {}                                  # (s, j) -> first abs tile
    gtiles = {}                                  # (s, j) -> tiles in group
    sup_tile0 = []                               # first abs tile of super
    p = 0                                        # in slots (always 128-aligned
    #                                              at group boundaries)
    for s in range(nsup):
        sup_tile0.append(p // 128)
        for j in range(NCHUNK):
            gtile0[(s, j)] = p // 128
            g0 = p
            for w in sup_ws[s]:
                A_wj[w, j] = p
                p += int(m_wj[w, j])
            p = ((p + 127) // 128) * 128
            gtiles[(s, j)] = (p - g0) // 128
    NT2 = p // 128                               # total stream tiles
    sup_tile0.append(NT2)

    # matmul list per super: (w, j, abs_tile) in emission order; per-window
    # start/stop counts
    mm_of_sup = []                               # list per super of (w,j,t)
    nmm_w = np.zeros(NWIN, np.int64)
    for s in range(nsup):
        lst = []
        for w in sup_ws[s]:
            for j in range(NCHUNK):
                if m_wj[w, j] == 0:
                    continue
                a, m = int(A_wj[w, j]), int(m_wj[w, j])
                for t in range(a // 128, (a + m + 127) // 128):
                    lst.append((w, j, t))
                    nmm_w[w] += 1
        mm_of_sup.append(lst)
    NMM = int(sum(len(x) for x in mm_of_sup))
    max_sup_mm = max(len(x) for x in mm_of_sup)
    max_sup_tiles = max(sup_tile0[s + 1] - sup_tile0[s] for s in range(nsup))

    # per-core slot data
    sidx = np.full((NCORE, NT2 * 128), -1, np.int16)
    dcol_slot = np.full((NCORE, NT2 * 128), -1, np.int64)
    for c in range(NCORE):
        for w in range(NWIN):
            for j in range(NCHUNK):
                cid = (c * NWIN + w) * NCHUNK + j
                e = order[starts[cid]:starts[cid + 1]]
                n = len(e)
                a = int(A_wj[w, j])
                sidx[c, a:a + n] = sloc[e]
                sidx[c, a + n:a + int(m_wj[w, j])] = 0   # interior pad
                dcol_slot[c, a:a + n] = dcol[e]
    # group tails keep idx=-1 (stripped at call end by the Q7)

    # per-matmul one-hot columns: [NMM, 128] values or -1
    dcol_mm = np.full((NCORE, NMM, 128), -1.0, F32)
    mm_idx = 0
    for s in range(nsup):
        for (w, j, t) in mm_of_sup[s]:
            a, m = int(A_wj[w, j]), int(m_wj[w, j])
            lo = max(a, t * 128)
            hi = min(a + m, (t + 1) * 128)
            for c in range(NCORE):
                seg = dcol_slot[c, lo:hi]
                dst_rows = np.arange(lo - t * 128, hi - t * 128)
                valid = seg >= 0
                dcol_mm[c, mm_idx, dst_rows[valid]] = seg[valid]
            mm_idx += 1
    assert mm_idx == NMM

    def idx_layout(a):      # [n] int16 -> [128, n//16]
        return np.tile(a.reshape(-1, 16).T, (8, 1))

    sidx_l = np.stack([idx_layout(sidx[c]) for c in range(NCORE)])
    dcol_l = np.ascontiguousarray(
        dcol_mm.transpose(0, 2, 1)).astype(BF16)    # [NCORE, 128, NMM]

    calls = []   # (chunk j, abs slot offset, n_idxs, super)
    for s in range(nsup):
        for j in range(NCHUNK):
            t0, nt = gtile0[(s, j)], gtiles[(s, j)]
            off = t0 * 128
            greal = int(sum(m_wj[w, j] for w in sup_ws[s]))   # real slots
            q = 0
            while q < greal:
                n = min(GCALL, ((greal - q + 15) // 16) * 16)
                calls.append((j, off + q, n, s))
                q += n

    feat = np.asarray(feat).astype(F32)
    featT = np.zeros((NCORE, 2, 128, NPPAD), BF16)
    nsp = np.zeros((NCORE, 128, NWIN), F32)
    nsdp = np.zeros((NCORE, 128, NWIN), F32)
    ndp = np.zeros((NCORE, 128, NWIN), F32)
    for c in range(NCORE):
        ft = feat[c * NP:(c + 1) * NP].astype(BF16).T   # [256, NP]
        featT[c, 0, :, :NP] = ft[0:128]
        featT[c, 1, :, :NP] = ft[128:256]
        for arr, dstp in ((ns, nsp), (nsd, nsdp), (nd, ndp)):
            v = np.zeros(NPPAD, F32)
            v[:NP] = arr[c * NP:(c + 1) * NP]
            dstp[c] = v.reshape(NWIN, 128).T

    ohchunk = min(32, max_sup_mm)
    consts = dict(
        w1=np.asarray(W1).astype(F32).astype(BF16),
        w2=np.asarray(W2).astype(F32).astype(BF16),
        w3p=np.pad(np.asarray(W3).astype(F32), ((0, 0), (0, 128 - D_OUT))).astype(BF16),
        iota=np.tile(np.arange(128, dtype=F32).astype(BF16)[None, :],
                     (128, ohchunk)),
    )
    sched = dict(NT2=NT2, NMM=NMM, calls=calls, mm_of_sup=mm_of_sup,
                 nmm_w=nmm_w, sup_tile0=sup_tile0, nsup=nsup, sup_ws=sup_ws,
                 max_sup_tiles=max_sup_tiles, max_sup_mm=max_sup_mm,
                 ohchunk=ohchunk)
    percore = dict(featT=featT, nsp=nsp, nsdp=nsdp, ndp=ndp,
                   sidx=sidx_l, dcol=dcol_l)
    return sched, consts, percore


def _build(sched):
    NT2 = sched["NT2"]; NMM = sched["NMM"]; calls = sched["calls"]
    mm_of_sup = sched["mm_of_sup"]; nmm_w = sched["nmm_w"]
    sup_tile0 = sched["sup_tile0"]; nsup = sched["nsup"]
    sup_ws = sched["sup_ws"]; max_sup_tiles = sched["max_sup_tiles"]
    max_sup_mm = sched["max_sup_mm"]; ohchunk = sched["ohchunk"]

    calls_by_sup = {}
    for c in calls:
        calls_by_sup.setdefault(c[3], []).append(c)
    # first matmul index (within super) per super, per window: for start/stop
    nc = bacc.Bacc("TRN2", target_bir_lowering=False, debug=False,
                   num_devices=NCORE, num_swdge_queues=NQUEUE)
    dt = mybir.dt

    featT_t = nc.dram_tensor("featT", [2, 128, NPPAD], dt.bfloat16,
                             kind="ExternalInput")
    w1_t = nc.dram_tensor("w1", [D_IN, D_H1], dt.bfloat16, kind="ExternalInput")
    w2_t = nc.dram_tensor("w2", [D_H1, D_H2], dt.bfloat16, kind="ExternalInput")
    w3_t = nc.dram_tensor("w3p", [D_H2, 128], dt.bfloat16, kind="ExternalInput")
    ns_t = nc.dram_tensor("nsp", [128, NWIN], dt.float32, kind="ExternalInput")
    nsd_t = nc.dram_tensor("nsdp", [128, NWIN], dt.float32, kind="ExternalInput")
    nd_t = nc.dram_tensor("ndp", [128, NWIN], dt.float32, kind="ExternalInput")
    sidx_t = nc.dram_tensor("sidx", [128, NT2 * 8], dt.int16, kind="ExternalInput")
    dcol_t = nc.dram_tensor("dcol", [128, NMM], dt.bfloat16, kind="ExternalInput")
    iota_t = nc.dram_tensor("iota", [128, ohchunk * 128], dt.bfloat16,
                            kind="ExternalInput")
    out_t = nc.dram_tensor("out", [NP, D_OUT], dt.float32, kind="ExternalOutput")

    qcount = [0]

    def next_queue():
        q = qcount[0] % NQUEUE
        qcount[0] += 1
        return q

    with tile.TileContext(nc) as tc:
        with (
            tc.tile_pool(name="const", bufs=1) as cpool,
            tc.tile_pool(name="hbuf", bufs=1) as hpool,
            tc.tile_pool(name="gb", bufs=3) as gpool,
            tc.tile_pool(name="ft", bufs=2) as ftpool,
            tc.tile_pool(name="work", bufs=3) as wpool,
            tc.tile_pool(name="oh", bufs=2) as ohpool,
            tc.tile_pool(name="ps", bufs=5, space="PSUM") as ppool,
            tc.tile_pool(name="pj", bufs=2, space="PSUM") as pjpool,
            tc.tile_pool(name="dram", bufs=1, space="DRAM") as dpool,
        ):
            w1a_s = cpool.tile([128, D_H1], dt.bfloat16)
            w1b_s = cpool.tile([128, D_H1], dt.bfloat16)
            w2_s = cpool.tile([D_H1, D_H2], dt.bfloat16)
            w3_s = cpool.tile([D_H2, 128], dt.bfloat16)
            ns_s = cpool.tile([128, NWIN], dt.float32)
            nsd_s = cpool.tile([128, NWIN], dt.float32)
            nd_s = cpool.tile([128, NWIN], dt.float32)
            sidx_s = cpool.tile([128, NT2 * 8], dt.int16)
            dcol_s = cpool.tile([128, NMM], dt.bfloat16)
            iota_s = cpool.tile([128, ohchunk * 128], dt.bfloat16)

            nc.sync.dma_start(w1a_s[:], w1_t.ap()[0:128, :])
            nc.sync.dma_start(w1b_s[:], w1_t.ap()[128:256, :])
            nc.sync.dma_start(w2_s[:], w2_t.ap())
            nc.sync.dma_start(w3_s[:], w3_t.ap())
            nc.sync.dma_start(ns_s[:], ns_t.ap())
            nc.sync.dma_start(nsd_s[:], nsd_t.ap())
            nc.sync.dma_start(nd_s[:], nd_t.ap())
            nc.sync.dma_start(sidx_s[:], sidx_t.ap())
            nc.sync.dma_start(dcol_s[:], dcol_t.ap())
            nc.sync.dma_start(iota_s[:], iota_t.ap())

            h_s = hpool.tile([128, NWIN * 128], dt.bfloat16)   # hT (feat x nodes)

            tins = [dpool.tile([NP, 128], dt.bfloat16, name=f"tin{L}")
                    for L in range(3)]
            tfulls = [[dpool.tile([CHUNK, 128], dt.bfloat16,
                                  name=f"tfull{L}_{q}", addr_space="Shared")
                       for q in range(NCHUNK)] for L in range(3)]

            # zero the two gather buffers once: stale bytes multiply with
            # one-hot zeros, so they must be finite (not NaN bit patterns)
            for _ in range(3):
                gz = gpool.tile([128, max_sup_tiles, 128], dt.bfloat16,
                                name="gsz", tag="gs",
                                padded_shape=[128, max_sup_tiles, 128])
                nc.vector.memset(gz[:], 0.0)

            def proj4(L, w0, wn, fta=None, ftb=None, k0=0):
                # wn (<=4) projection windows into one PSUM bank, one wide
                # scale instr, per-window tin writes
                ppj = pjpool.tile([128, wn * 128], dt.float32, name=f"pj{L}",
                                  tag="pj", padded_shape=[128, 512])
                for k in range(wn):
                    w = w0 + k
                    osl = ppj[:, k * 128:(k + 1) * 128]
                    if L == 0:
                        nc.tensor.matmul(osl,
                                         lhsT=fta[:, (k0 + k) * 128:(k0 + k + 1) * 128],
                                         rhs=w1a_s[:], start=True, stop=False)
                        nc.tensor.matmul(osl,
                                         lhsT=ftb[:, (k0 + k) * 128:(k0 + k + 1) * 128],
                                         rhs=w1b_s[:], start=False, stop=True)
                    else:
                        rhs = w2_s if L == 1 else w3_s
                        nc.tensor.matmul(osl,
                                         lhsT=h_s[:, w * 128:(w + 1) * 128],
                                         rhs=rhs[:], start=True, stop=True)
                scal = ns_s if L == 0 else nsd_s
                ssl = scal[:, w0:w0 + wn]
                sbc = AP(ssl.tensor, ssl.offset, list(ssl.ap) + [[0, 128]])
                pbf = wpool.tile([128, wn * 128], dt.bfloat16, name="pbf",
                                 tag="pbf", padded_shape=[128, 512])
                nc.vector.tensor_tensor(out=pbf[:, 0:wn * 128],
                                        in0=ppj[:, 0:wn * 128], in1=sbc,
                                        op=mybir.AluOpType.mult)
                for k in range(wn):
                    w = w0 + k
                    wsz = min(128, NP - w * 128)
                    nc.sync.dma_start(tins[L][w * 128:w * 128 + wsz, :],
                                      pbf[:wsz, k * 128:(k + 1) * 128])

            def ag(L, q):
                nc.gpsimd.collective_compute(
                    "AllGather", mybir.AluOpType.bypass,
                    replica_groups=[list(range(NCORE))],
                    ins=[tins[L][q * QROWS:(q + 1) * QROWS, :].opt()],
                    outs=[tfulls[L][q][:].opt()],
                )

            def agg(L, s):
                stile0 = sup_tile0[s]
                stiles = sup_tile0[s + 1] - stile0
                smms = mm_of_sup[s]
                nmm_s = len(smms)
                mm0 = sum(len(mm_of_sup[ss]) for ss in range(s))
                gs = gpool.tile([128, stiles, 128], dt.bfloat16,
                                name=f"gs{L}_{s}", tag="gs",
                                padded_shape=[128, max_sup_tiles, 128])
                for (j, off, n, cs) in calls_by_sup.get(s, []):
                    rel = off // 128 - stile0
                    nc.gpsimd.dma_gather(
                        gs[:, rel:rel + (n + 127) // 128, :],
                        tfulls[L][j][:],
                        sidx_s[:, off // 16:(off + n) // 16],
                        n, n, 128,
                        queue_num=next_queue(),
                    )
                oh = ohpool.tile([128, nmm_s * 128], dt.bfloat16,
                                 name=f"oh{L}", tag="oh",
                                 padded_shape=[128, max_sup_mm * 128])
                q = 0
                while q < nmm_s:
                    nb = min(ohchunk, nmm_s - q)
                    dsl = dcol_s[:, mm0 + q:mm0 + q + nb]
                    bcast = AP(dsl.tensor, dsl.offset,
                               list(dsl.ap) + [[0, 128]])
                    nc.vector.tensor_tensor(
                        out=oh[:, q * 128:(q + nb) * 128],
                        in0=iota_s[:, 0:nb * 128],
                        in1=bcast,
                        op=mybir.AluOpType.is_equal)
                    q += nb
                # per-window PSUM accumulation over this super's matmul list
                aps_of_w = {}
                done_of_w = {}
                for mi, (w, j, t) in enumerate(smms):
                    if w not in aps_of_w:
                        aps_of_w[w] = ppool.tile([128, 128], dt.float32,
                                                 name=f"ap{L}", tag="pp")
                        done_of_w[w] = 0
                    aps = aps_of_w[w]
                    k = done_of_w[w]
                    ohsl = oh[:, mi * 128:(mi + 1) * 128]
                    first, last = k == 0, k == int(nmm_w[w]) - 1
                    if L < 2:
                        nc.tensor.matmul(aps[:], lhsT=gs[:, t - stile0, :],
                                         rhs=ohsl, start=first, stop=last)
                    else:
                        nc.tensor.matmul(aps[:], lhsT=ohsl,
                                         rhs=gs[:, t - stile0, :],
                                         start=first, stop=last)
                    done_of_w[w] = k + 1
                    if not last:
                        continue
                    if L < 2:
                        nc.scalar.activation(
                            h_s[:, w * 128:(w + 1) * 128], aps[:],
                            mybir.ActivationFunctionType.Relu)
                    else:
                        wsz = min(128, NP - w * 128)
                        ob = wpool.tile([128, D_OUT], dt.float32, name="ob",
                                        tag="ob")
                        nc.vector.tensor_scalar(
                            out=ob[:], in0=aps[:, 0:D_OUT],
                            scalar1=nd_s[:, w:w + 1], scalar2=None,
                            op0=mybir.AluOpType.mult)
                        nc.sync.dma_start(
                            out_t.ap()[w * 128:w * 128 + wsz, :], ob[:wsz, :])

            # ---- L0 projection (staged featT slices) + quarter-AGs ----
            agq = 0
            for w0 in range(0, NWIN, FTW):
                wn = min(FTW, NWIN - w0)
                fta = ftpool.tile([128, wn * 128], dt.bfloat16, name="fta",
                                  tag="fta", padded_shape=[128, FTW * 128])
                ftb = ftpool.tile([128, wn * 128], dt.bfloat16, name="ftb",
                                  tag="ftb", padded_shape=[128, FTW * 128])
                nc.sync.dma_start(
                    fta[:], featT_t.ap()[0, :, w0 * 128:(w0 + wn) * 128])
                nc.sync.dma_start(
                    ftb[:], featT_t.ap()[1, :, w0 * 128:(w0 + wn) * 128])
                for g0 in range(0, wn, 4):
                    gn = min(4, wn - g0)
                    proj4(0, w0 + g0, gn, fta, ftb, g0)
                    while agq < NCHUNK and w0 + g0 + gn - 1 >= AGWIN[agq]:
                        ag(0, agq)
                        agq += 1

            # ---- pipelined layers ----
            agsup = [min(nsup - 1, (AGWIN[q] // SUPW) + 2) for q in range(NCHUNK)]
            for L in range(3):
                nagq = 0
                for s in range(nsup):
                    agg(L, s)
                    if L < 2:
                        proj4(L + 1, sup_ws[s][0], len(sup_ws[s]))
                        while nagq < NCHUNK and s >= agsup[nagq]:
                            ag(L + 1, nagq)
                            nagq += 1

    nc.compile()
    return nc


def _in_map(consts, percore, c):
    return {
        "featT": percore["featT"][c],
        "w1": consts["w1"], "w2": consts["w2"], "w3p": consts["w3p"],
        "nsp": percore["nsp"][c], "nsdp": percore["nsdp"][c],
        "ndp": percore["ndp"][c],
        "sidx": percore["sidx"][c], "dcol": percore["dcol"][c],
        "iota": consts["iota"],
    }


def kernel(feat, W1, b1, W2, b2, W3, b3, src, dst):
    sched, consts, percore = _host_prep(feat, W1, b1, W2, b2, W3, b3, src, dst)
    nc = _build(sched)
    in_maps = [_in_map(consts, percore, c) for c in range(NCORE)]
    res = run_bass_kernel_spmd(nc, in_maps, core_ids=list(range(NCORE)))
    out = np.concatenate([res.results[c]["out"][:NP] for c in range(NCORE)],
                         axis=0)
    return np.ascontiguousarray(out.astype(np.float32))


# revision 16
# speedup vs baseline: 1.1511x; 1.0099x over previous
"""3-layer GCN (DGL GraphConv, norm='both') on 8 Trainium2 NeuronCores.

v4: descriptor-minimized, software-pipelined SPMD single-NEFF design.
  - Nodes partitioned contiguously: core c owns rows [c*12500, (c+1)*12500).
  - Per layer: project own nodes on PE (bf16) -> [12500,128] bf16 shard;
    AllGather split into FOUR quarter-collectives that fire as soon as their
    projection windows complete (table chunk q = concat over cores of local
    rows [q*3125,(q+1)*3125), keeping int16 gather indices valid).
  - Per-edge SWDGE dma_gather fetches source rows; calls round-robin over 4
    SWDGE queues so all four Q7 core pairs generate descriptors in parallel.
  - Gather stream layout: cells (dst-window, src-chunk) sized to the
    cross-core max count and concatenated per (super-group, chunk) WITHOUT
    per-cell rounding (only the group tail pads to a tile). Tiles may span
    window boundaries; each (window, chunk, tile) matmul gets its own
    one-hot column block with -1 (no match) marking rows of other windows.
    This cuts gathered rows ~12% vs per-cell 128-rounding.
  - Segment-sum by dst via one-hot matmul accumulation in PSUM over 128-dst
    windows; one-hot built in wide DVE is_equal instrs.
  - Next-layer projection windows are emitted right after each super-group's
    aggregation, keeping PE/DMA/Q7 busy across layer boundaries.
  - Layers 1-2 keep h transposed ([feat x nodes]); the in-degree norm is
    deferred into the next projection's per-row scale (zero bias asserted).
  - Host (numpy) does index-only prep (degrees, bucketing, sorting, padding
    to a core-uniform static schedule - SPMD needs identical instruction
    streams on all 8 cores).
"""

import numpy as np
import ml_dtypes

import concourse.bacc as bacc
import concourse.bass as bass
import concourse.mybir as mybir
import concourse.tile as tile
from concourse.bass import AP
from concourse.bass_utils import run_bass_kernel_spmd

BF16 = ml_dtypes.bfloat16
F32 = np.float32

N_NODES = 100000
D_IN, D_H1, D_H2, D_OUT = 256, 128, 128, 64
NCORE = 8
NP = N_NODES // NCORE        # 12500 nodes per core
NWIN = (NP + 127) // 128     # 98 windows (last holds 84)
NPPAD = NWIN * 128           # 12544
NCHUNK = 4
QROWS = NP // NCHUNK         # 3125 rows per quarter-AG per core
CHUNK = QROWS * NCORE        # 25000 rows per gather chunk
GCALL = 1024                 # idxs per dma_gather call
SUPW = 4                     # windows per gather super-group
NQUEUE = 4                   # SWDGE queues (Q7 core pairs)
FTW = 8                      # windows per staged featT slice (L0)
# last proj window needed by quarter-AG q
AGWIN = [((QROWS * (q + 1) + 127) // 128) - 1 for q in range(NCHUNK)]


def _host_prep(feat, W1, b1, W2, b2, W3, b3, src, dst):
    src = np.asarray(src).astype(np.int64)
    dst = np.asarray(dst).astype(np.int64)
    for b in (b1, b2, b3):
        assert np.max(np.abs(np.asarray(b))) == 0.0, \
            "nonzero bias needs the undeferred-nd path"

    deg_out = np.bincount(src, minlength=N_NODES).astype(F32)
    deg_in = np.bincount(dst, minlength=N_NODES).astype(F32)
    ns = 1.0 / np.sqrt(np.maximum(deg_out, 1.0))
    nd = 1.0 / np.sqrt(np.maximum(deg_in, 1.0))
    nsd = ns * nd

    core = dst // NP
    dloc = dst % NP
    win = dloc // 128
    c_src = src // NP
    r_src = src % NP
    chunk = r_src // QROWS
    sloc = (c_src * QROWS + (r_src % QROWS)).astype(np.int16)
    dcol = (dloc % 128).astype(np.int32)

    ncell = NCORE * NWIN * NCHUNK
    cell = ((core * NWIN + win) * NCHUNK + chunk).astype(np.int64)
    order = np.argsort(cell * CHUNK + sloc, kind="stable")
    counts = np.bincount(cell, minlength=ncell).reshape(NCORE, NWIN, NCHUNK)
    m_wj = counts.max(axis=0).astype(np.int64)      # uniform cell sizes

    starts = np.zeros(ncell + 1, np.int64)
    np.cumsum(np.bincount(cell, minlength=ncell), out=starts[1:])

    nsup = (NWIN + SUPW - 1) // SUPW
    sup_ws = [list(range(s * SUPW, min((s + 1) * SUPW, NWIN)))
              for s in range(nsup)]

    # slot layout: per (super, chunk) group: cells concatenated (no per-cell
    # rounding), group padded to a tile boundary.
    A_wj = np.zeros((NWIN, NCHUNK), np.int64)   # absolute slot offset of cell
    gtile0 = {}                                  # (s, j) -> first abs tile
    gtiles = {}                                  # (s, j) -> tiles in group
    sup_tile0 = []                               # first abs tile of super
    p = 0                                        # in slots (always 128-aligned
    #                                              at group boundaries)
    for s in range(nsup):
        sup_tile0.append(p // 128)
        for j in range(NCHUNK):
            gtile0[(s, j)] = p // 128
            g0 = p
            for w in sup_ws[s]:
                A_wj[w, j] = p
                p += int(m_wj[w, j])
            p = ((p + 127) // 128) * 128
            gtiles[(s, j)] = (p - g0) // 128
    NT2 = p // 128                               # total stream tiles
    sup_tile0.append(NT2)

    # matmul list per super: (w, j, abs_tile) in emission order; per-window
    # start/stop counts
    mm_of_sup = []                               # list per super of (w,j,t)
    nmm_w = np.zeros(NWIN, np.int64)
    for s in range(nsup):
        lst = []
        for w in sup_ws[s]:
            for j in range(NCHUNK):
                if m_wj[w, j] == 0:
                    continue
                a, m = int(A_wj[w, j]), int(m_wj[w, j])
                for t in range(a // 128, (a + m + 127) // 128):
                    lst.append((w, j, t))
                    nmm_w[w] += 1
        mm_of_sup.append(lst)
    NMM = int(sum(len(x) for x in mm_of_sup))
    max_sup_mm = max(len(x) for x in mm_of_sup)
    max_sup_tiles = max(sup_tile0[s + 1] - sup_tile0[s] for s in range(nsup))

    # per-core slot data
    sidx = np.full((NCORE, NT2 * 128), -1, np.int16)
    dcol_slot = np.full((NCORE, NT2 * 128), -1, np.int64)
    for c in range(NCORE):
        for w in range(NWIN):
            for j in range(NCHUNK):
                cid = (c * NWIN + w) * NCHUNK + j
                e = order[starts[cid]:starts[cid + 1]]
                n = len(e)
                a = int(A_wj[w, j])
                sidx[c, a:a + n] = sloc[e]
                sidx[c, a + n:a + int(m_wj[w, j])] = 0   # interior pad
                dcol_slot[c, a:a + n] = dcol[e]
    # group tails keep idx=-1 (stripped at call end by the Q7)

    # per-matmul one-hot columns: [NMM, 128] values or -1
    dcol_mm = np.full((NCORE, NMM, 128), -1.0, F32)
    mm_idx = 0
    for s in range(nsup):
        for (w, j, t) in mm_of_sup[s]:
            a, m = int(A_wj[w, j]), int(m_wj[w, j])
            lo = max(a, t * 128)
            hi = min(a + m, (t + 1) * 128)
            for c in range(NCORE):
                seg = dcol_slot[c, lo:hi]
                dst_rows = np.arange(lo - t * 128, hi - t * 128)
                valid = seg >= 0
                dcol_mm[c, mm_idx, dst_rows[valid]] = seg[valid]
            mm_idx += 1
    assert mm_idx == NMM

    def idx_layout(a):      # [n] int16 -> [128, n//16]
        return np.tile(a.reshape(-1, 16).T, (8, 1))

    sidx_l = np.stack([idx_layout(sidx[c]) for c in range(NCORE)])
    dcol_l = np.ascontiguousarray(
        dcol_mm.transpose(0, 2, 1)).astype(BF16)    # [NCORE, 128, NMM]

    calls = []   # (chunk j, abs slot offset, n_idxs, super)
    for s in range(nsup):
        for j in range(NCHUNK):
            t0, nt = gtile0[(s, j)], gtiles[(s, j)]
            off = t0 * 128
            greal = int(sum(m_wj[w, j] for w in sup_ws[s]))   # real slots
            q = 0
            while q < greal:
                n = min(GCALL, ((greal - q + 15) // 16) * 16)
                calls.append((j, off + q, n, s))
                q += n

    feat = np.asarray(feat).astype(F32)
    featT = np.zeros((NCORE, 2, 128, NPPAD), BF16)
    nsp = np.zeros((NCORE, 128, NWIN), F32)
    nsdp = np.zeros((NCORE, 128, NWIN), F32)
    ndp = np.zeros((NCORE, 128, NWIN), F32)
    for c in range(NCORE):
        ft = feat[c * NP:(c + 1) * NP].astype(BF16).T   # [256, NP]
        featT[c, 0, :, :NP] = ft[0:128]
        featT[c, 1, :, :NP] = ft[128:256]
        for arr, dstp in ((ns, nsp), (nsd, nsdp), (nd, ndp)):
            v = np.zeros(NPPAD, F32)
            v[:NP] = arr[c * NP:(c + 1) * NP]
            dstp[c] = v.reshape(NWIN, 128).T

    ohchunk = min(32, max_sup_mm)
    consts = dict(
        w1=np.asarray(W1).astype(F32).astype(BF16),
        w2=np.asarray(W2).astype(F32).astype(BF16),
        w3p=np.pad(np.asarray(W3).astype(F32), ((0, 0), (0, 128 - D_OUT))).astype(BF16),
        iota=np.tile(np.arange(128, dtype=F32).astype(BF16)[None, :],
                     (128, ohchunk)),
    )
    sched = dict(NT2=NT2, NMM=NMM, calls=calls, mm_of_sup=mm_of_sup,
                 nmm_w=nmm_w, sup_tile0=sup_tile0, nsup=nsup, sup_ws=sup_ws,
                 max_sup_tiles=max_sup_tiles, max_sup_mm=max_sup_mm,
                 ohchunk=ohchunk)
    percore = dict(featT=featT, nsp=nsp, nsdp=nsdp, ndp=ndp,
                   sidx=sidx_l, dcol=dcol_l)
    return sched, consts, percore


def _build(sched):
    NT2 = sched["NT2"]; NMM = sched["NMM"]; calls = sched["calls"]
    mm_of_sup = sched["mm_of_sup"]; nmm_w = sched["nmm_w"]
    sup_tile0 = sched["sup_tile0"]; nsup = sched["nsup"]
    sup_ws = sched["sup_ws"]; max_sup_tiles = sched["max_sup_tiles"]
    max_sup_mm = sched["max_sup_mm"]; ohchunk = sched["ohchunk"]

    calls_by_sup = {}
    for c in calls:
        calls_by_sup.setdefault(c[3], []).append(c)
    # first matmul index (within super) per super, per window: for start/stop
    nc = bacc.Bacc("TRN2", target_bir_lowering=False, debug=False,
                   num_devices=NCORE, num_swdge_queues=NQUEUE)
    dt = mybir.dt

    featT_t = nc.dram_tensor("featT", [2, 128, NPPAD], dt.bfloat16,
                             kind="ExternalInput")
    w1_t = nc.dram_tensor("w1", [D_IN, D_H1], dt.bfloat16, kind="ExternalInput")
    w2_t = nc.dram_tensor("w2", [D_H1, D_H2], dt.bfloat16, kind="ExternalInput")
    w3_t = nc.dram_tensor("w3p", [D_H2, 128], dt.bfloat16, kind="ExternalInput")
    ns_t = nc.dram_tensor("nsp", [128, NWIN], dt.float32, kind="ExternalInput")
    nsd_t = nc.dram_tensor("nsdp", [128, NWIN], dt.float32, kind="ExternalInput")
    nd_t = nc.dram_tensor("ndp", [128, NWIN], dt.float32, kind="ExternalInput")
    sidx_t = nc.dram_tensor("sidx", [128, NT2 * 8], dt.int16, kind="ExternalInput")
    dcol_t = nc.dram_tensor("dcol", [128, NMM], dt.bfloat16, kind="ExternalInput")
    iota_t = nc.dram_tensor("iota", [128, ohchunk * 128], dt.bfloat16,
                            kind="ExternalInput")
    out_t = nc.dram_tensor("out", [NP, D_OUT], dt.float32, kind="ExternalOutput")

    qcount = [0]

    def next_queue():
        q = qcount[0] % NQUEUE
        qcount[0] += 1
        return q

    with tile.TileContext(nc) as tc:
        with (
            tc.tile_pool(name="const", bufs=1) as cpool,
            tc.tile_pool(name="hbuf", bufs=1) as hpool,
            tc.tile_pool(name="gb", bufs=3) as gpool,
            tc.tile_pool(name="ft", bufs=2) as ftpool,
            tc.tile_pool(name="work", bufs=3) as wpool,
            tc.tile_pool(name="oh", bufs=2) as ohpool,
            tc.tile_pool(name="ps", bufs=5, space="PSUM") as ppool,
            tc.tile_pool(name="pj", bufs=2, space="PSUM") as pjpool,
            tc.tile_pool(name="dram", bufs=1, space="DRAM") as dpool,
        ):
            w1a_s = cpool.tile([128, D_H1], dt.bfloat16)
            w1b_s = cpool.tile([128, D_H1], dt.bfloat16)
            w2_s = cpool.tile([D_H1, D_H2], dt.bfloat16)
            w3_s = cpool.tile([D_H2, 128], dt.bfloat16)
            ns_s = cpool.tile([128, NWIN], dt.float32)
            nsd_s = cpool.tile([128, NWIN], dt.float32)
            nd_s = cpool.tile([128, NWIN], dt.float32)
            sidx_s = cpool.tile([128, NT2 * 8], dt.int16)
            dcol_s = cpool.tile([128, NMM], dt.bfloat16)
            iota_s = cpool.tile([128, ohchunk * 128], dt.bfloat16)

            nc.sync.dma_start(w1a_s[:], w1_t.ap()[0:128, :])
            nc.sync.dma_start(w1b_s[:], w1_t.ap()[128:256, :])
            nc.sync.dma_start(w2_s[:], w2_t.ap())
            nc.sync.dma_start(w3_s[:], w3_t.ap())
            nc.sync.dma_start(ns_s[:], ns_t.ap())
            nc.sync.dma_start(nsd_s[:], nsd_t.ap())
            nc.sync.dma_start(nd_s[:], nd_t.ap())
            nc.sync.dma_start(sidx_s[:], sidx_t.ap())
            nc.sync.dma_start(dcol_s[:], dcol_t.ap())
            nc.sync.dma_start(iota_s[:], iota_t.ap())

            h_s = hpool.tile([128, NWIN * 128], dt.bfloat16)   # hT (feat x nodes)

            tins = [dpool.tile([NP, 128], dt.bfloat16, name=f"tin{L}")
                    for L in range(3)]
            tfulls = [[dpool.tile([CHUNK, 128], dt.bfloat16,
                                  name=f"tfull{L}_{q}", addr_space="Shared")
                       for q in range(NCHUNK)] for L in range(3)]

            # zero the two gather buffers once: stale bytes multiply with
            # one-hot zeros, so they must be finite (not NaN bit patterns)
            for _ in range(3):
                gz = gpool.tile([128, max_sup_tiles, 128], dt.bfloat16,
                                name="gsz", tag="gs",
                                padded_shape=[128, max_sup_tiles, 128])
                nc.vector.memset(gz[:], 0.0)

            def proj4(L, w0, wn, fta=None, ftb=None, k0=0):
                # wn (<=4) projection windows into one PSUM bank, one wide
                # scale instr, per-window tin writes
                ppj = pjpool.tile([128, wn * 128], dt.float32, name=f"pj{L}",
                                  tag="pj", padded_shape=[128, 512])
                for k in range(wn):
                    w = w0 + k
                    osl = ppj[:, k * 128:(k + 1) * 128]
                    if L == 0:
                        nc.tensor.matmul(osl,
                                         lhsT=fta[:, (k0 + k) * 128:(k0 + k + 1) * 128],
                                         rhs=w1a_s[:], start=True, stop=False)
                        nc.tensor.matmul(osl,
                                         lhsT=ftb[:, (k0 + k) * 128:(k0 + k + 1) * 128],
                                         rhs=w1b_s[:], start=False, stop=True)
                    else:
                        rhs = w2_s if L == 1 else w3_s
                        nc.tensor.matmul(osl,
                                         lhsT=h_s[:, w * 128:(w + 1) * 128],
                                         rhs=rhs[:], start=True, stop=True)
                scal = ns_s if L == 0 else nsd_s
                ssl = scal[:, w0:w0 + wn]
                sbc = AP(ssl.tensor, ssl.offset, list(ssl.ap) + [[0, 128]])
                pbf = wpool.tile([128, wn * 128], dt.bfloat16, name="pbf",
                                 tag="pbf", padded_shape=[128, 512])
                nc.vector.tensor_tensor(out=pbf[:, 0:wn * 128],
                                        in0=ppj[:, 0:wn * 128], in1=sbc,
                                        op=mybir.AluOpType.mult)
                for k in range(wn):
                    w = w0 + k
                    wsz = min(128, NP - w * 128)
                    nc.sync.dma_start(tins[L][w * 128:w * 128 + wsz, :],
                                      pbf[:wsz, k * 128:(k + 1) * 128])

            def ag(L, q):
                nc.gpsimd.collective_compute(
                    "AllGather", mybir.AluOpType.bypass,
                    replica_groups=[list(range(NCORE))],
                    ins=[tins[L][q * QROWS:(q + 1) * QROWS, :].opt()],
                    outs=[tfulls[L][q][:].opt()],
                )

            def agg(L, s, ag_hooks=None):
                stile0 = sup_tile0[s]
                stiles = sup_tile0[s + 1] - stile0
                smms = mm_of_sup[s]
                nmm_s = len(smms)
                mm0 = sum(len(mm_of_sup[ss]) for ss in range(s))
                gs = gpool.tile([128, stiles, 128], dt.bfloat16,
                                name=f"gs{L}_{s}", tag="gs",
                                padded_shape=[128, max_sup_tiles, 128])
                for (j, off, n, cs) in calls_by_sup.get(s, []):
                    if ag_hooks and j in ag_hooks:
                        ag_hooks.pop(j)()
                    rel = off // 128 - stile0
                    nc.gpsimd.dma_gather(
                        gs[:, rel:rel + (n + 127) // 128, :],
                        tfulls[L][j][:],
                        sidx_s[:, off // 16:(off + n) // 16],
                        n, n, 128,
                        queue_num=next_queue(),
                    )
                oh = ohpool.tile([128, nmm_s * 128], dt.bfloat16,
                                 name=f"oh{L}", tag="oh",
                                 padded_shape=[128, max_sup_mm * 128])
                q = 0
                while q < nmm_s:
                    nb = min(ohchunk, nmm_s - q)
                    dsl = dcol_s[:, mm0 + q:mm0 + q + nb]
                    bcast = AP(dsl.tensor, dsl.offset,
                               list(dsl.ap) + [[0, 128]])
                    nc.vector.tensor_tensor(
                        out=oh[:, q * 128:(q + nb) * 128],
                        in0=iota_s[:, 0:nb * 128],
                        in1=bcast,
                        op=mybir.AluOpType.is_equal)
                    q += nb
                # per-window PSUM accumulation over this super's matmul list
                aps_of_w = {}
                done_of_w = {}
                for mi, (w, j, t) in enumerate(smms):
                    if w not in aps_of_w:
                        aps_of_w[w] = ppool.tile([128, 128], dt.float32,
                                                 name=f"ap{L}", tag="pp")
                        done_of_w[w] = 0
                    aps = aps_of_w[w]
                    k = done_of_w[w]
                    ohsl = oh[:, mi * 128:(mi + 1) * 128]
                    first, last = k == 0, k == int(nmm_w[w]) - 1
                    if L < 2:
                        nc.tensor.matmul(aps[:], lhsT=gs[:, t - stile0, :],
                                         rhs=ohsl, start=first, stop=last)
                    else:
                        nc.tensor.matmul(aps[:, 0:D_OUT], lhsT=ohsl,
                                         rhs=gs[:, t - stile0, 0:D_OUT],
                                         start=first, stop=last)
                    done_of_w[w] = k + 1
                    if not last:
                        continue
                    if L < 2:
                        nc.scalar.activation(
                            h_s[:, w * 128:(w + 1) * 128], aps[:],
                            mybir.ActivationFunctionType.Relu)
                    else:
                        wsz = min(128, NP - w * 128)
                        ob = wpool.tile([128, D_OUT], dt.float32, name="ob",
                                        tag="ob")
                        nc.vector.tensor_scalar(
                            out=ob[:], in0=aps[:, 0:D_OUT],
                            scalar1=nd_s[:, w:w + 1], scalar2=None,
                            op0=mybir.AluOpType.mult)
                        nc.sync.dma_start(
                            out_t.ap()[w * 128:w * 128 + wsz, :], ob[:wsz, :])

            # ---- L0 projection (staged featT slices) + quarter-AGs ----
            agq = 0
            for w0 in range(0, NWIN, FTW):
                wn = min(FTW, NWIN - w0)
                fta = ftpool.tile([128, wn * 128], dt.bfloat16, name="fta",
                                  tag="fta", padded_shape=[128, FTW * 128])
                ftb = ftpool.tile([128, wn * 128], dt.bfloat16, name="ftb",
                                  tag="ftb", padded_shape=[128, FTW * 128])
                nc.sync.dma_start(
                    fta[:], featT_t.ap()[0, :, w0 * 128:(w0 + wn) * 128])
                nc.sync.dma_start(
                    ftb[:], featT_t.ap()[1, :, w0 * 128:(w0 + wn) * 128])
                for g0 in range(0, wn, 4):
                    gn = min(4, wn - g0)
                    proj4(0, w0 + g0, gn, fta, ftb, g0)
                    while agq < 1 and w0 + g0 + gn - 1 >= AGWIN[agq]:
                        ag(0, agq)
                        agq += 1

            # ---- pipelined layers ----
            agsup = [min(nsup - 1, (AGWIN[q] // SUPW) + 2) for q in range(NCHUNK)]
            for L in range(3):
                hooks = ({q: (lambda qq=q: ag(0, qq)) for q in (1, 2, 3)}
                         if L == 0 else None)
                nagq = 0
                for s in range(nsup):
                    agg(L, s, ag_hooks=hooks if s == 0 else None)
                    if L < 2:
                        proj4(L + 1, sup_ws[s][0], len(sup_ws[s]))
                        while nagq < NCHUNK and s >= agsup[nagq]:
                            ag(L + 1, nagq)
                            nagq += 1

    nc.compile()
    return nc


def _in_map(consts, percore, c):
    return {
        "featT": percore["featT"][c],
        "w1": consts["w1"], "w2": consts["w2"], "w3p": consts["w3p"],
        "nsp": percore["nsp"][c], "nsdp": percore["nsdp"][c],
        "ndp": percore["ndp"][c],
        "sidx": percore["sidx"][c], "dcol": percore["dcol"][c],
        "iota": consts["iota"],
    }


def kernel(feat, W1, b1, W2, b2, W3, b3, src, dst):
    sched, consts, percore = _host_prep(feat, W1, b1, W2, b2, W3, b3, src, dst)
    nc = _build(sched)
    in_maps = [_in_map(consts, percore, c) for c in range(NCORE)]
    res = run_bass_kernel_spmd(nc, in_maps, core_ids=list(range(NCORE)))
    out = np.concatenate([res.results[c]["out"][:NP] for c in range(NCORE)],
                         axis=0)
    return np.ascontiguousarray(out.astype(np.float32))


# revision 17
# speedup vs baseline: 1.1685x; 1.0151x over previous
"""3-layer GCN (DGL GraphConv, norm='both') on 8 Trainium2 NeuronCores.

v4: descriptor-minimized, software-pipelined SPMD single-NEFF design.
  - Nodes partitioned contiguously: core c owns rows [c*12500, (c+1)*12500).
  - Per layer: project own nodes on PE (bf16) -> [12500,128] bf16 shard;
    AllGather split into FOUR quarter-collectives that fire as soon as their
    projection windows complete (table chunk q = concat over cores of local
    rows [q*3125,(q+1)*3125), keeping int16 gather indices valid).
  - Per-edge SWDGE dma_gather fetches source rows; calls round-robin over 4
    SWDGE queues so all four Q7 core pairs generate descriptors in parallel.
  - Gather stream layout: cells (dst-window, src-chunk) sized to the
    cross-core max count and concatenated per (super-group, chunk) WITHOUT
    per-cell rounding (only the group tail pads to a tile). Tiles may span
    window boundaries; each (window, chunk, tile) matmul gets its own
    one-hot column block with -1 (no match) marking rows of other windows.
    This cuts gathered rows ~12% vs per-cell 128-rounding.
  - Segment-sum by dst via one-hot matmul accumulation in PSUM over 128-dst
    windows; one-hot built in wide DVE is_equal instrs.
  - Next-layer projection windows are emitted right after each super-group's
    aggregation, keeping PE/DMA/Q7 busy across layer boundaries.
  - Layers 1-2 keep h transposed ([feat x nodes]); the in-degree norm is
    deferred into the next projection's per-row scale (zero bias asserted).
  - Host (numpy) does index-only prep (degrees, bucketing, sorting, padding
    to a core-uniform static schedule - SPMD needs identical instruction
    streams on all 8 cores).
"""

import numpy as np
import ml_dtypes

import concourse.bacc as bacc
import concourse.bass as bass
import concourse.mybir as mybir
import concourse.tile as tile
from concourse.bass import AP
from concourse.bass_utils import run_bass_kernel_spmd

BF16 = ml_dtypes.bfloat16
F32 = np.float32

N_NODES = 100000
D_IN, D_H1, D_H2, D_OUT = 256, 128, 128, 64
NCORE = 8
NP = N_NODES // NCORE        # 12500 nodes per core
NWIN = (NP + 127) // 128     # 98 windows (last holds 84)
NPPAD = NWIN * 128           # 12544
NCHUNK = 4
QROWS = NP // NCHUNK         # 3125 rows per quarter-AG per core
CHUNK = QROWS * NCORE        # 25000 rows per gather chunk
GCALL = 1024                 # idxs per dma_gather call
SUPW = 4                     # windows per gather super-group
NQUEUE = 4                   # SWDGE queues (Q7 core pairs)
FTW = 8                      # windows per staged featT slice (L0)
# last proj window needed by quarter-AG q
AGWIN = [((QROWS * (q + 1) + 127) // 128) - 1 for q in range(NCHUNK)]


def _host_prep(feat, W1, b1, W2, b2, W3, b3, src, dst):
    src = np.asarray(src).astype(np.int64)
    dst = np.asarray(dst).astype(np.int64)
    for b in (b1, b2, b3):
        assert np.max(np.abs(np.asarray(b))) == 0.0, \
            "nonzero bias needs the undeferred-nd path"

    deg_out = np.bincount(src, minlength=N_NODES).astype(F32)
    deg_in = np.bincount(dst, minlength=N_NODES).astype(F32)
    ns = 1.0 / np.sqrt(np.maximum(deg_out, 1.0))
    nd = 1.0 / np.sqrt(np.maximum(deg_in, 1.0))
    nsd = ns * nd

    core = dst // NP
    dloc = dst % NP
    win = dloc // 128
    c_src = src // NP
    r_src = src % NP
    chunk = r_src // QROWS
    sloc = (c_src * QROWS + (r_src % QROWS)).astype(np.int16)
    dcol = (dloc % 128).astype(np.int32)

    ncell = NCORE * NWIN * NCHUNK
    cell = ((core * NWIN + win) * NCHUNK + chunk).astype(np.int64)
    order = np.argsort(cell * CHUNK + sloc, kind="stable")
    counts = np.bincount(cell, minlength=ncell).reshape(NCORE, NWIN, NCHUNK)
    m_wj = counts.max(axis=0).astype(np.int64)      # uniform cell sizes

    starts = np.zeros(ncell + 1, np.int64)
    np.cumsum(np.bincount(cell, minlength=ncell), out=starts[1:])

    nsup = (NWIN + SUPW - 1) // SUPW
    sup_ws = [list(range(s * SUPW, min((s + 1) * SUPW, NWIN)))
              for s in range(nsup)]

    # slot layout: per (super, chunk) group: cells concatenated (no per-cell
    # rounding), group padded to a tile boundary.
    A_wj = np.zeros((NWIN, NCHUNK), np.int64)   # absolute slot offset of cell
    gtile0 = {}                                  # (s, j) -> first abs tile
    gtiles = {}                                  # (s, j) -> tiles in group
    sup_tile0 = []                               # first abs tile of super
    p = 0                                        # in slots (always 128-aligned
    #                                              at group boundaries)
    for s in range(nsup):
        sup_tile0.append(p // 128)
        for j in range(NCHUNK):
            gtile0[(s, j)] = p // 128
            g0 = p
            for w in sup_ws[s]:
                A_wj[w, j] = p
                p += int(m_wj[w, j])
            p = ((p + 127) // 128) * 128
            gtiles[(s, j)] = (p - g0) // 128
    NT2 = p // 128                               # total stream tiles
    sup_tile0.append(NT2)

    # matmul list per super: (w, j, abs_tile) in emission order; per-window
    # start/stop counts
    mm_of_sup = []                               # list per super of (w,j,t)
    nmm_w = np.zeros(NWIN, np.int64)
    for s in range(nsup):
        lst = []
        for w in sup_ws[s]:
            for j in range(NCHUNK):
                if m_wj[w, j] == 0:
                    continue
                a, m = int(A_wj[w, j]), int(m_wj[w, j])
                for t in range(a // 128, (a + m + 127) // 128):
                    lst.append((w, j, t))
                    nmm_w[w] += 1
        mm_of_sup.append(lst)
    NMM = int(sum(len(x) for x in mm_of_sup))
    max_sup_mm = max(len(x) for x in mm_of_sup)
    max_sup_tiles = max(sup_tile0[s + 1] - sup_tile0[s] for s in range(nsup))

    # per-core slot data
    sidx = np.full((NCORE, NT2 * 128), -1, np.int16)
    dcol_slot = np.full((NCORE, NT2 * 128), -1, np.int64)
    for c in range(NCORE):
        for w in range(NWIN):
            for j in range(NCHUNK):
                cid = (c * NWIN + w) * NCHUNK + j
                e = order[starts[cid]:starts[cid + 1]]
                n = len(e)
                a = int(A_wj[w, j])
                sidx[c, a:a + n] = sloc[e]
                sidx[c, a + n:a + int(m_wj[w, j])] = 0   # interior pad
                dcol_slot[c, a:a + n] = dcol[e]
    # group tails keep idx=-1 (stripped at call end by the Q7)

    # per-matmul one-hot columns: [NMM, 128] values or -1
    dcol_mm = np.full((NCORE, NMM, 128), -1.0, F32)
    mm_idx = 0
    for s in range(nsup):
        for (w, j, t) in mm_of_sup[s]:
            a, m = int(A_wj[w, j]), int(m_wj[w, j])
            lo = max(a, t * 128)
            hi = min(a + m, (t + 1) * 128)
            for c in range(NCORE):
                seg = dcol_slot[c, lo:hi]
                dst_rows = np.arange(lo - t * 128, hi - t * 128)
                valid = seg >= 0
                dcol_mm[c, mm_idx, dst_rows[valid]] = seg[valid]
            mm_idx += 1
    assert mm_idx == NMM

    def idx_layout(a):      # [n] int16 -> [128, n//16]
        return np.tile(a.reshape(-1, 16).T, (8, 1))

    sidx_l = np.stack([idx_layout(sidx[c]) for c in range(NCORE)])
    dcol_l = np.ascontiguousarray(
        dcol_mm.transpose(0, 2, 1)).astype(BF16)    # [NCORE, 128, NMM]

    calls = []   # (chunk j, abs slot offset, n_idxs, super)
    for s in range(nsup):
        for j in range(NCHUNK):
            t0, nt = gtile0[(s, j)], gtiles[(s, j)]
            off = t0 * 128
            greal = int(sum(m_wj[w, j] for w in sup_ws[s]))   # real slots
            q = 0
            while q < greal:
                n = min(GCALL, ((greal - q + 15) // 16) * 16)
                calls.append((j, off + q, n, s))
                q += n

    feat = np.asarray(feat).astype(F32)
    featT = np.zeros((NCORE, 2, 128, NPPAD), BF16)
    nsp = np.zeros((NCORE, 128, NWIN), F32)
    nsdp = np.zeros((NCORE, 128, NWIN), F32)
    ndp = np.zeros((NCORE, 128, NWIN), F32)
    for c in range(NCORE):
        ft = feat[c * NP:(c + 1) * NP].astype(BF16).T   # [256, NP]
        featT[c, 0, :, :NP] = ft[0:128]
        featT[c, 1, :, :NP] = ft[128:256]
        for arr, dstp in ((ns, nsp), (nsd, nsdp), (nd, ndp)):
            v = np.zeros(NPPAD, F32)
            v[:NP] = arr[c * NP:(c + 1) * NP]
            dstp[c] = v.reshape(NWIN, 128).T

    ohchunk = min(32, max_sup_mm)
    consts = dict(
        w1=np.asarray(W1).astype(F32).astype(BF16),
        w2=np.asarray(W2).astype(F32).astype(BF16),
        w3p=np.pad(np.asarray(W3).astype(F32), ((0, 0), (0, 128 - D_OUT))).astype(BF16),
        iota=np.tile(np.arange(128, dtype=F32).astype(BF16)[None, :],
                     (128, ohchunk)),
    )
    sched = dict(NT2=NT2, NMM=NMM, calls=calls, mm_of_sup=mm_of_sup,
                 nmm_w=nmm_w, sup_tile0=sup_tile0, nsup=nsup, sup_ws=sup_ws,
                 max_sup_tiles=max_sup_tiles, max_sup_mm=max_sup_mm,
                 ohchunk=ohchunk)
    percore = dict(featT=featT, nsp=nsp, nsdp=nsdp, ndp=ndp,
                   sidx=sidx_l, dcol=dcol_l)
    return sched, consts, percore


def _build(sched):
    NT2 = sched["NT2"]; NMM = sched["NMM"]; calls = sched["calls"]
    mm_of_sup = sched["mm_of_sup"]; nmm_w = sched["nmm_w"]
    sup_tile0 = sched["sup_tile0"]; nsup = sched["nsup"]
    sup_ws = sched["sup_ws"]; max_sup_tiles = sched["max_sup_tiles"]
    max_sup_mm = sched["max_sup_mm"]; ohchunk = sched["ohchunk"]

    calls_by_sup = {}
    for c in calls:
        calls_by_sup.setdefault(c[3], []).append(c)
    # first matmul index (within super) per super, per window: for start/stop
    nc = bacc.Bacc("TRN2", target_bir_lowering=False, debug=False,
                   num_devices=NCORE, num_swdge_queues=NQUEUE)
    dt = mybir.dt

    featT_t = nc.dram_tensor("featT", [2, 128, NPPAD], dt.bfloat16,
                             kind="ExternalInput")
    w1_t = nc.dram_tensor("w1", [D_IN, D_H1], dt.bfloat16, kind="ExternalInput")
    w2_t = nc.dram_tensor("w2", [D_H1, D_H2], dt.bfloat16, kind="ExternalInput")
    w3_t = nc.dram_tensor("w3p", [D_H2, 128], dt.bfloat16, kind="ExternalInput")
    ns_t = nc.dram_tensor("nsp", [128, NWIN], dt.float32, kind="ExternalInput")
    nsd_t = nc.dram_tensor("nsdp", [128, NWIN], dt.float32, kind="ExternalInput")
    nd_t = nc.dram_tensor("ndp", [128, NWIN], dt.float32, kind="ExternalInput")
    sidx_t = nc.dram_tensor("sidx", [128, NT2 * 8], dt.int16, kind="ExternalInput")
    dcol_t = nc.dram_tensor("dcol", [128, NMM], dt.bfloat16, kind="ExternalInput")
    iota_t = nc.dram_tensor("iota", [128, ohchunk * 128], dt.bfloat16,
                            kind="ExternalInput")
    out_t = nc.dram_tensor("out", [NP, D_OUT], dt.float32, kind="ExternalOutput")

    qcount = [0]

    def next_queue():
        q = qcount[0] % NQUEUE
        qcount[0] += 1
        return q

    with tile.TileContext(nc) as tc:
        with (
            tc.tile_pool(name="const", bufs=1) as cpool,
            tc.tile_pool(name="hbuf", bufs=1) as hpool,
            tc.tile_pool(name="gb", bufs=3) as gpool,
            tc.tile_pool(name="ft", bufs=2) as ftpool,
            tc.tile_pool(name="work", bufs=3) as wpool,
            tc.tile_pool(name="oh", bufs=3) as ohpool,
            tc.tile_pool(name="ps", bufs=6, space="PSUM") as ppool,
            tc.tile_pool(name="pj", bufs=2, space="PSUM") as pjpool,
            tc.tile_pool(name="dram", bufs=1, space="DRAM") as dpool,
        ):
            w1a_s = cpool.tile([128, D_H1], dt.bfloat16)
            w1b_s = cpool.tile([128, D_H1], dt.bfloat16)
            w2_s = cpool.tile([D_H1, D_H2], dt.bfloat16)
            w3_s = cpool.tile([D_H2, 128], dt.bfloat16)
            ns_s = cpool.tile([128, NWIN], dt.float32)
            nsd_s = cpool.tile([128, NWIN], dt.float32)
            nd_s = cpool.tile([128, NWIN], dt.float32)
            sidx_s = cpool.tile([128, NT2 * 8], dt.int16)
            dcol_s = cpool.tile([128, NMM], dt.bfloat16)
            iota_s = cpool.tile([128, ohchunk * 128], dt.bfloat16)

            nc.sync.dma_start(w1a_s[:], w1_t.ap()[0:128, :])
            nc.sync.dma_start(w1b_s[:], w1_t.ap()[128:256, :])
            nc.sync.dma_start(w2_s[:], w2_t.ap())
            nc.sync.dma_start(w3_s[:], w3_t.ap())
            nc.sync.dma_start(ns_s[:], ns_t.ap())
            nc.sync.dma_start(nsd_s[:], nsd_t.ap())
            nc.sync.dma_start(nd_s[:], nd_t.ap())
            nc.sync.dma_start(sidx_s[:], sidx_t.ap())
            nc.sync.dma_start(dcol_s[:], dcol_t.ap())
            nc.sync.dma_start(iota_s[:], iota_t.ap())

            h_s = hpool.tile([128, NWIN * 128], dt.bfloat16)   # hT (feat x nodes)

            tins = [dpool.tile([NP, 128], dt.bfloat16, name=f"tin{L}")
                    for L in range(3)]
            tfulls = [[dpool.tile([CHUNK, 128], dt.bfloat16,
                                  name=f"tfull{L}_{q}", addr_space="Shared")
                       for q in range(NCHUNK)] for L in range(3)]

            # zero the two gather buffers once: stale bytes multiply with
            # one-hot zeros, so they must be finite (not NaN bit patterns)
            for _ in range(3):
                gz = gpool.tile([128, max_sup_tiles, 128], dt.bfloat16,
                                name="gsz", tag="gs",
                                padded_shape=[128, max_sup_tiles, 128])
                nc.vector.memset(gz[:], 0.0)

            def proj4(L, w0, wn, fta=None, ftb=None, k0=0):
                # wn (<=4) projection windows into one PSUM bank, one wide
                # scale instr, per-window tin writes
                ppj = pjpool.tile([128, wn * 128], dt.float32, name=f"pj{L}",
                                  tag="pj", padded_shape=[128, 512])
                for k in range(wn):
                    w = w0 + k
                    osl = ppj[:, k * 128:(k + 1) * 128]
                    if L == 0:
                        nc.tensor.matmul(osl,
                                         lhsT=fta[:, (k0 + k) * 128:(k0 + k + 1) * 128],
                                         rhs=w1a_s[:], start=True, stop=False)
                        nc.tensor.matmul(osl,
                                         lhsT=ftb[:, (k0 + k) * 128:(k0 + k + 1) * 128],
                                         rhs=w1b_s[:], start=False, stop=True)
                    else:
                        rhs = w2_s if L == 1 else w3_s
                        nc.tensor.matmul(osl,
                                         lhsT=h_s[:, w * 128:(w + 1) * 128],
                                         rhs=rhs[:], start=True, stop=True)
                scal = ns_s if L == 0 else nsd_s
                ssl = scal[:, w0:w0 + wn]
                sbc = AP(ssl.tensor, ssl.offset, list(ssl.ap) + [[0, 128]])
                pbf = wpool.tile([128, wn * 128], dt.bfloat16, name="pbf",
                                 tag="pbf", padded_shape=[128, 512])
                nc.vector.tensor_tensor(out=pbf[:, 0:wn * 128],
                                        in0=ppj[:, 0:wn * 128], in1=sbc,
                                        op=mybir.AluOpType.mult)
                for k in range(wn):
                    w = w0 + k
                    wsz = min(128, NP - w * 128)
                    nc.sync.dma_start(tins[L][w * 128:w * 128 + wsz, :],
                                      pbf[:wsz, k * 128:(k + 1) * 128])

            def ag(L, q):
                nc.gpsimd.collective_compute(
                    "AllGather", mybir.AluOpType.bypass,
                    replica_groups=[list(range(NCORE))],
                    ins=[tins[L][q * QROWS:(q + 1) * QROWS, :].opt()],
                    outs=[tfulls[L][q][:].opt()],
                )

            def agg(L, s, ag_hooks=None):
                stile0 = sup_tile0[s]
                stiles = sup_tile0[s + 1] - stile0
                smms = mm_of_sup[s]
                nmm_s = len(smms)
                mm0 = sum(len(mm_of_sup[ss]) for ss in range(s))
                gs = gpool.tile([128, stiles, 128], dt.bfloat16,
                                name=f"gs{L}_{s}", tag="gs",
                                padded_shape=[128, max_sup_tiles, 128])
                for (j, off, n, cs) in calls_by_sup.get(s, []):
                    if ag_hooks and j in ag_hooks:
                        ag_hooks.pop(j)()
                    rel = off // 128 - stile0
                    nc.gpsimd.dma_gather(
                        gs[:, rel:rel + (n + 127) // 128, :],
                        tfulls[L][j][:],
                        sidx_s[:, off // 16:(off + n) // 16],
                        n, n, 128,
                        queue_num=next_queue(),
                    )
                oh = ohpool.tile([128, nmm_s * 128], dt.bfloat16,
                                 name=f"oh{L}", tag="oh",
                                 padded_shape=[128, max_sup_mm * 128])
                q = 0
                while q < nmm_s:
                    nb = min(ohchunk, nmm_s - q)
                    dsl = dcol_s[:, mm0 + q:mm0 + q + nb]
                    bcast = AP(dsl.tensor, dsl.offset,
                               list(dsl.ap) + [[0, 128]])
                    nc.vector.tensor_tensor(
                        out=oh[:, q * 128:(q + nb) * 128],
                        in0=iota_s[:, 0:nb * 128],
                        in1=bcast,
                        op=mybir.AluOpType.is_equal)
                    q += nb
                # per-window PSUM accumulation over this super's matmul list
                aps_of_w = {}
                done_of_w = {}
                for mi, (w, j, t) in enumerate(smms):
                    if w not in aps_of_w:
                        aps_of_w[w] = ppool.tile([128, 128], dt.float32,
                                                 name=f"ap{L}", tag="pp")
                        done_of_w[w] = 0
                    aps = aps_of_w[w]
                    k = done_of_w[w]
                    ohsl = oh[:, mi * 128:(mi + 1) * 128]
                    first, last = k == 0, k == int(nmm_w[w]) - 1
                    if L < 2:
                        nc.tensor.matmul(aps[:], lhsT=gs[:, t - stile0, :],
                                         rhs=ohsl, start=first, stop=last)
                    else:
                        nc.tensor.matmul(aps[:, 0:D_OUT], lhsT=ohsl,
                                         rhs=gs[:, t - stile0, 0:D_OUT],
                                         start=first, stop=last)
                    done_of_w[w] = k + 1
                    if not last:
                        continue
                    if L < 2:
                        nc.scalar.activation(
                            h_s[:, w * 128:(w + 1) * 128], aps[:],
                            mybir.ActivationFunctionType.Relu)
                    else:
                        wsz = min(128, NP - w * 128)
                        ob = wpool.tile([128, D_OUT], dt.float32, name="ob",
                                        tag="ob")
                        nc.vector.tensor_scalar(
                            out=ob[:], in0=aps[:, 0:D_OUT],
                            scalar1=nd_s[:, w:w + 1], scalar2=None,
                            op0=mybir.AluOpType.mult)
                        nc.sync.dma_start(
                            out_t.ap()[w * 128:w * 128 + wsz, :], ob[:wsz, :])

            # ---- L0 projection (staged featT slices) + quarter-AGs ----
            agq = 0
            for w0 in range(0, NWIN, FTW):
                wn = min(FTW, NWIN - w0)
                fta = ftpool.tile([128, wn * 128], dt.bfloat16, name="fta",
                                  tag="fta", padded_shape=[128, FTW * 128])
                ftb = ftpool.tile([128, wn * 128], dt.bfloat16, name="ftb",
                                  tag="ftb", padded_shape=[128, FTW * 128])
                nc.sync.dma_start(
                    fta[:], featT_t.ap()[0, :, w0 * 128:(w0 + wn) * 128])
                nc.sync.dma_start(
                    ftb[:], featT_t.ap()[1, :, w0 * 128:(w0 + wn) * 128])
                for g0 in range(0, wn, 4):
                    gn = min(4, wn - g0)
                    proj4(0, w0 + g0, gn, fta, ftb, g0)
                    while agq < 1 and w0 + g0 + gn - 1 >= AGWIN[agq]:
                        ag(0, agq)
                        agq += 1

            # ---- pipelined layers ----
            agsup = [min(nsup - 1, (AGWIN[q] // SUPW) + 2) for q in range(NCHUNK)]
            for L in range(3):
                hooks = ({q: (lambda qq=q: ag(0, qq)) for q in (1, 2, 3)}
                         if L == 0 else None)
                nagq = 0
                for s in range(nsup):
                    agg(L, s, ag_hooks=hooks if s == 0 else None)
                    if L < 2:
                        proj4(L + 1, sup_ws[s][0], len(sup_ws[s]))
                        while nagq < NCHUNK and s >= agsup[nagq]:
                            ag(L + 1, nagq)
                            nagq += 1

    nc.compile()
    return nc


def _in_map(consts, percore, c):
    return {
        "featT": percore["featT"][c],
        "w1": consts["w1"], "w2": consts["w2"], "w3p": consts["w3p"],
        "nsp": percore["nsp"][c], "nsdp": percore["nsdp"][c],
        "ndp": percore["ndp"][c],
        "sidx": percore["sidx"][c], "dcol": percore["dcol"][c],
        "iota": consts["iota"],
    }


def kernel(feat, W1, b1, W2, b2, W3, b3, src, dst):
    sched, consts, percore = _host_prep(feat, W1, b1, W2, b2, W3, b3, src, dst)
    nc = _build(sched)
    in_maps = [_in_map(consts, percore, c) for c in range(NCORE)]
    res = run_bass_kernel_spmd(nc, in_maps, core_ids=list(range(NCORE)))
    out = np.concatenate([res.results[c]["out"][:NP] for c in range(NCORE)],
                         axis=0)
    return np.ascontiguousarray(out.astype(np.float32))


# revision 18
# speedup vs baseline: 1.1918x; 1.0199x over previous
"""3-layer GCN (DGL GraphConv, norm='both') on 8 Trainium2 NeuronCores.

v4: descriptor-minimized, software-pipelined SPMD single-NEFF design.
  - Nodes partitioned contiguously: core c owns rows [c*12500, (c+1)*12500).
  - Per layer: project own nodes on PE (bf16) -> [12500,128] bf16 shard;
    AllGather split into FOUR quarter-collectives that fire as soon as their
    projection windows complete (table chunk q = concat over cores of local
    rows [q*3125,(q+1)*3125), keeping int16 gather indices valid).
  - Per-edge SWDGE dma_gather fetches source rows; calls round-robin over 4
    SWDGE queues so all four Q7 core pairs generate descriptors in parallel.
  - Gather stream layout: cells (dst-window, src-chunk) sized to the
    cross-core max count and concatenated per (super-group, chunk) WITHOUT
    per-cell rounding (only the group tail pads to a tile). Tiles may span
    window boundaries; each (window, chunk, tile) matmul gets its own
    one-hot column block with -1 (no match) marking rows of other windows.
    This cuts gathered rows ~12% vs per-cell 128-rounding.
  - Segment-sum by dst via one-hot matmul accumulation in PSUM over 128-dst
    windows; one-hot built in wide DVE is_equal instrs.
  - Next-layer projection windows are emitted right after each super-group's
    aggregation, keeping PE/DMA/Q7 busy across layer boundaries.
  - Layers 1-2 keep h transposed ([feat x nodes]); the in-degree norm is
    deferred into the next projection's per-row scale (zero bias asserted).
  - Host (numpy) does index-only prep (degrees, bucketing, sorting, padding
    to a core-uniform static schedule - SPMD needs identical instruction
    streams on all 8 cores).
"""

import numpy as np
import ml_dtypes

import concourse.bacc as bacc
import concourse.bass as bass
import concourse.mybir as mybir
import concourse.tile as tile
from concourse.bass import AP
from concourse.bass_utils import run_bass_kernel_spmd

BF16 = ml_dtypes.bfloat16
F32 = np.float32

N_NODES = 100000
D_IN, D_H1, D_H2, D_OUT = 256, 128, 128, 64
NCORE = 8
NP = N_NODES // NCORE        # 12500 nodes per core
NWIN = (NP + 127) // 128     # 98 windows (last holds 84)
NPPAD = NWIN * 128           # 12544
NCHUNK = 4
# uneven quarters: the LAST quarter-AG gates each layer boundary (it needs
# the final projection window), so keep it small to shrink that exposed
# latency. max sloc = 8*3584-1 = 28671 stays within int16.
QROWS_L = [3584, 3584, 3584, NP - 3 * 3584]      # per-quarter rows per core
QCUM = [0]
for _q in QROWS_L:
    QCUM.append(QCUM[-1] + _q)
CHUNK_L = [q * NCORE for q in QROWS_L]           # gather chunk sizes
GCALL = 1024                 # idxs per dma_gather call
SUPW = 4                     # windows per gather super-group
NQUEUE = 4                   # SWDGE queues (Q7 core pairs)
FTW = 8                      # windows per staged featT slice (L0)
# last proj window needed by quarter-AG q
AGWIN = [((QCUM[q + 1] + 127) // 128) - 1 for q in range(NCHUNK)]


def _host_prep(feat, W1, b1, W2, b2, W3, b3, src, dst):
    src = np.asarray(src).astype(np.int64)
    dst = np.asarray(dst).astype(np.int64)
    for b in (b1, b2, b3):
        assert np.max(np.abs(np.asarray(b))) == 0.0, \
            "nonzero bias needs the undeferred-nd path"

    deg_out = np.bincount(src, minlength=N_NODES).astype(F32)
    deg_in = np.bincount(dst, minlength=N_NODES).astype(F32)
    ns = 1.0 / np.sqrt(np.maximum(deg_out, 1.0))
    nd = 1.0 / np.sqrt(np.maximum(deg_in, 1.0))
    nsd = ns * nd

    core = dst // NP
    dloc = dst % NP
    win = dloc // 128
    c_src = src // NP
    r_src = src % NP
    chunk = np.searchsorted(np.array(QCUM[1:]), r_src, side="right")
    qr = np.array([QROWS_L[q] for q in chunk])
    q0 = np.array([QCUM[q] for q in chunk])
    sloc = (c_src * qr + (r_src - q0)).astype(np.int16)
    dcol = (dloc % 128).astype(np.int32)

    ncell = NCORE * NWIN * NCHUNK
    cell = ((core * NWIN + win) * NCHUNK + chunk).astype(np.int64)
    order = np.argsort(cell * 32768 + sloc, kind="stable")
    counts = np.bincount(cell, minlength=ncell).reshape(NCORE, NWIN, NCHUNK)
    m_wj = counts.max(axis=0).astype(np.int64)      # uniform cell sizes

    starts = np.zeros(ncell + 1, np.int64)
    np.cumsum(np.bincount(cell, minlength=ncell), out=starts[1:])

    nsup = (NWIN + SUPW - 1) // SUPW
    sup_ws = [list(range(s * SUPW, min((s + 1) * SUPW, NWIN)))
              for s in range(nsup)]

    # slot layout: per (super, chunk) group: cells concatenated (no per-cell
    # rounding), group padded to a tile boundary.
    A_wj = np.zeros((NWIN, NCHUNK), np.int64)   # absolute slot offset of cell
    gtile0 = {}                                  # (s, j) -> first abs tile
    gtiles = {}                                  # (s, j) -> tiles in group
    sup_tile0 = []                               # first abs tile of super
    p = 0                                        # in slots (always 128-aligned
    #                                              at group boundaries)
    for s in range(nsup):
        sup_tile0.append(p // 128)
        for j in range(NCHUNK):
            gtile0[(s, j)] = p // 128
            g0 = p
            for w in sup_ws[s]:
                A_wj[w, j] = p
                p += int(m_wj[w, j])
            p = ((p + 127) // 128) * 128
            gtiles[(s, j)] = (p - g0) // 128
    NT2 = p // 128                               # total stream tiles
    sup_tile0.append(NT2)

    # matmul list per super: (w, j, abs_tile) in emission order; per-window
    # start/stop counts
    mm_of_sup = []                               # list per super of (w,j,t)
    nmm_w = np.zeros(NWIN, np.int64)
    for s in range(nsup):
        lst = []
        for w in sup_ws[s]:
            for j in range(NCHUNK):
                if m_wj[w, j] == 0:
                    continue
                a, m = int(A_wj[w, j]), int(m_wj[w, j])
                for t in range(a // 128, (a + m + 127) // 128):
                    lst.append((w, j, t))
                    nmm_w[w] += 1
        mm_of_sup.append(lst)
    NMM = int(sum(len(x) for x in mm_of_sup))
    max_sup_mm = max(len(x) for x in mm_of_sup)
    max_sup_tiles = max(sup_tile0[s + 1] - sup_tile0[s] for s in range(nsup))

    # per-core slot data
    sidx = np.full((NCORE, NT2 * 128), -1, np.int16)
    dcol_slot = np.full((NCORE, NT2 * 128), -1, np.int64)
    for c in range(NCORE):
        for w in range(NWIN):
            for j in range(NCHUNK):
                cid = (c * NWIN + w) * NCHUNK + j
                e = order[starts[cid]:starts[cid + 1]]
                n = len(e)
                a = int(A_wj[w, j])
                sidx[c, a:a + n] = sloc[e]
                sidx[c, a + n:a + int(m_wj[w, j])] = 0   # interior pad
                dcol_slot[c, a:a + n] = dcol[e]
    # group tails keep idx=-1 (stripped at call end by the Q7)

    # per-matmul one-hot columns: [NMM, 128] values or -1
    dcol_mm = np.full((NCORE, NMM, 128), -1.0, F32)
    mm_idx = 0
    for s in range(nsup):
        for (w, j, t) in mm_of_sup[s]:
            a, m = int(A_wj[w, j]), int(m_wj[w, j])
            lo = max(a, t * 128)
            hi = min(a + m, (t + 1) * 128)
            for c in range(NCORE):
                seg = dcol_slot[c, lo:hi]
                dst_rows = np.arange(lo - t * 128, hi - t * 128)
                valid = seg >= 0
                dcol_mm[c, mm_idx, dst_rows[valid]] = seg[valid]
            mm_idx += 1
    assert mm_idx == NMM

    def idx_layout(a):      # [n] int16 -> [128, n//16]
        return np.tile(a.reshape(-1, 16).T, (8, 1))

    sidx_l = np.stack([idx_layout(sidx[c]) for c in range(NCORE)])
    dcol_l = np.ascontiguousarray(
        dcol_mm.transpose(0, 2, 1)).astype(BF16)    # [NCORE, 128, NMM]

    calls = []   # (chunk j, abs slot offset, n_idxs, super)
    for s in range(nsup):
        for j in range(NCHUNK):
            t0, nt = gtile0[(s, j)], gtiles[(s, j)]
            off = t0 * 128
            greal = int(sum(m_wj[w, j] for w in sup_ws[s]))   # real slots
            q = 0
            while q < greal:
                n = min(GCALL, ((greal - q + 15) // 16) * 16)
                calls.append((j, off + q, n, s))
                q += n

    feat = np.asarray(feat).astype(F32)
    featT = np.zeros((NCORE, 2, 128, NPPAD), BF16)
    nsp = np.zeros((NCORE, 128, NWIN), F32)
    nsdp = np.zeros((NCORE, 128, NWIN), F32)
    ndp = np.zeros((NCORE, 128, NWIN), F32)
    for c in range(NCORE):
        ft = feat[c * NP:(c + 1) * NP].astype(BF16).T   # [256, NP]
        featT[c, 0, :, :NP] = ft[0:128]
        featT[c, 1, :, :NP] = ft[128:256]
        for arr, dstp in ((ns, nsp), (nsd, nsdp), (nd, ndp)):
            v = np.zeros(NPPAD, F32)
            v[:NP] = arr[c * NP:(c + 1) * NP]
            dstp[c] = v.reshape(NWIN, 128).T

    ohchunk = min(32, max_sup_mm)
    consts = dict(
        w1=np.asarray(W1).astype(F32).astype(BF16),
        w2=np.asarray(W2).astype(F32).astype(BF16),
        w3p=np.pad(np.asarray(W3).astype(F32), ((0, 0), (0, 128 - D_OUT))).astype(BF16),
        iota=np.tile(np.arange(128, dtype=F32).astype(BF16)[None, :],
                     (128, ohchunk)),
    )
    sched = dict(NT2=NT2, NMM=NMM, calls=calls, mm_of_sup=mm_of_sup,
                 nmm_w=nmm_w, sup_tile0=sup_tile0, nsup=nsup, sup_ws=sup_ws,
                 max_sup_tiles=max_sup_tiles, max_sup_mm=max_sup_mm,
                 ohchunk=ohchunk)
    percore = dict(featT=featT, nsp=nsp, nsdp=nsdp, ndp=ndp,
                   sidx=sidx_l, dcol=dcol_l)
    return sched, consts, percore


def _build(sched):
    NT2 = sched["NT2"]; NMM = sched["NMM"]; calls = sched["calls"]
    mm_of_sup = sched["mm_of_sup"]; nmm_w = sched["nmm_w"]
    sup_tile0 = sched["sup_tile0"]; nsup = sched["nsup"]
    sup_ws = sched["sup_ws"]; max_sup_tiles = sched["max_sup_tiles"]
    max_sup_mm = sched["max_sup_mm"]; ohchunk = sched["ohchunk"]

    calls_by_sup = {}
    for c in calls:
        calls_by_sup.setdefault(c[3], []).append(c)
    # first matmul index (within super) per super, per window: for start/stop
    nc = bacc.Bacc("TRN2", target_bir_lowering=False, debug=False,
                   num_devices=NCORE, num_swdge_queues=NQUEUE)
    dt = mybir.dt

    featT_t = nc.dram_tensor("featT", [2, 128, NPPAD], dt.bfloat16,
                             kind="ExternalInput")
    w1_t = nc.dram_tensor("w1", [D_IN, D_H1], dt.bfloat16, kind="ExternalInput")
    w2_t = nc.dram_tensor("w2", [D_H1, D_H2], dt.bfloat16, kind="ExternalInput")
    w3_t = nc.dram_tensor("w3p", [D_H2, 128], dt.bfloat16, kind="ExternalInput")
    ns_t = nc.dram_tensor("nsp", [128, NWIN], dt.float32, kind="ExternalInput")
    nsd_t = nc.dram_tensor("nsdp", [128, NWIN], dt.float32, kind="ExternalInput")
    nd_t = nc.dram_tensor("ndp", [128, NWIN], dt.float32, kind="ExternalInput")
    sidx_t = nc.dram_tensor("sidx", [128, NT2 * 8], dt.int16, kind="ExternalInput")
    dcol_t = nc.dram_tensor("dcol", [128, NMM], dt.bfloat16, kind="ExternalInput")
    iota_t = nc.dram_tensor("iota", [128, ohchunk * 128], dt.bfloat16,
                            kind="ExternalInput")
    out_t = nc.dram_tensor("out", [NP, D_OUT], dt.float32, kind="ExternalOutput")

    qcount = [0]

    def next_queue():
        q = qcount[0] % NQUEUE
        qcount[0] += 1
        return q

    with tile.TileContext(nc) as tc:
        with (
            tc.tile_pool(name="const", bufs=1) as cpool,
            tc.tile_pool(name="hbuf", bufs=1) as hpool,
            tc.tile_pool(name="gb", bufs=3) as gpool,
            tc.tile_pool(name="ft", bufs=2) as ftpool,
            tc.tile_pool(name="work", bufs=3) as wpool,
            tc.tile_pool(name="oh", bufs=3) as ohpool,
            tc.tile_pool(name="ps", bufs=6, space="PSUM") as ppool,
            tc.tile_pool(name="pj", bufs=2, space="PSUM") as pjpool,
            tc.tile_pool(name="dram", bufs=1, space="DRAM") as dpool,
        ):
            w1a_s = cpool.tile([128, D_H1], dt.bfloat16)
            w1b_s = cpool.tile([128, D_H1], dt.bfloat16)
            w2_s = cpool.tile([D_H1, D_H2], dt.bfloat16)
            w3_s = cpool.tile([D_H2, 128], dt.bfloat16)
            ns_s = cpool.tile([128, NWIN], dt.float32)
            nsd_s = cpool.tile([128, NWIN], dt.float32)
            nd_s = cpool.tile([128, NWIN], dt.float32)
            sidx_s = cpool.tile([128, NT2 * 8], dt.int16)
            dcol_s = cpool.tile([128, NMM], dt.bfloat16)
            iota_s = cpool.tile([128, ohchunk * 128], dt.bfloat16)

            nc.sync.dma_start(w1a_s[:], w1_t.ap()[0:128, :])
            nc.sync.dma_start(w1b_s[:], w1_t.ap()[128:256, :])
            nc.sync.dma_start(w2_s[:], w2_t.ap())
            nc.sync.dma_start(w3_s[:], w3_t.ap())
            nc.sync.dma_start(ns_s[:], ns_t.ap())
            nc.sync.dma_start(nsd_s[:], nsd_t.ap())
            nc.sync.dma_start(nd_s[:], nd_t.ap())
            nc.sync.dma_start(sidx_s[:], sidx_t.ap())
            nc.sync.dma_start(dcol_s[:], dcol_t.ap())
            nc.sync.dma_start(iota_s[:], iota_t.ap())

            h_s = hpool.tile([128, NWIN * 128], dt.bfloat16)   # hT (feat x nodes)

            tins = [dpool.tile([NP, 128], dt.bfloat16, name=f"tin{L}")
                    for L in range(3)]
            tfulls = [[dpool.tile([CHUNK_L[q], 128], dt.bfloat16,
                                  name=f"tfull{L}_{q}", addr_space="Shared")
                       for q in range(NCHUNK)] for L in range(3)]

            # zero the two gather buffers once: stale bytes multiply with
            # one-hot zeros, so they must be finite (not NaN bit patterns)
            for _ in range(3):
                gz = gpool.tile([128, max_sup_tiles, 128], dt.bfloat16,
                                name="gsz", tag="gs",
                                padded_shape=[128, max_sup_tiles, 128])
                nc.vector.memset(gz[:], 0.0)

            def proj4(L, w0, wn, fta=None, ftb=None, k0=0):
                # wn (<=4) projection windows into one PSUM bank, one wide
                # scale instr, per-window tin writes
                ppj = pjpool.tile([128, wn * 128], dt.float32, name=f"pj{L}",
                                  tag="pj", padded_shape=[128, 512])
                for k in range(wn):
                    w = w0 + k
                    osl = ppj[:, k * 128:(k + 1) * 128]
                    if L == 0:
                        nc.tensor.matmul(osl,
                                         lhsT=fta[:, (k0 + k) * 128:(k0 + k + 1) * 128],
                                         rhs=w1a_s[:], start=True, stop=False)
                        nc.tensor.matmul(osl,
                                         lhsT=ftb[:, (k0 + k) * 128:(k0 + k + 1) * 128],
                                         rhs=w1b_s[:], start=False, stop=True)
                    else:
                        rhs = w2_s if L == 1 else w3_s
                        nc.tensor.matmul(osl,
                                         lhsT=h_s[:, w * 128:(w + 1) * 128],
                                         rhs=rhs[:], start=True, stop=True)
                scal = ns_s if L == 0 else nsd_s
                ssl = scal[:, w0:w0 + wn]
                sbc = AP(ssl.tensor, ssl.offset, list(ssl.ap) + [[0, 128]])
                pbf = wpool.tile([128, wn * 128], dt.bfloat16, name="pbf",
                                 tag="pbf", padded_shape=[128, 512])
                nc.vector.tensor_tensor(out=pbf[:, 0:wn * 128],
                                        in0=ppj[:, 0:wn * 128], in1=sbc,
                                        op=mybir.AluOpType.mult)
                for k in range(wn):
                    w = w0 + k
                    wsz = min(128, NP - w * 128)
                    nc.sync.dma_start(tins[L][w * 128:w * 128 + wsz, :],
                                      pbf[:wsz, k * 128:(k + 1) * 128])

            def ag(L, q):
                nc.gpsimd.collective_compute(
                    "AllGather", mybir.AluOpType.bypass,
                    replica_groups=[list(range(NCORE))],
                    ins=[tins[L][QCUM[q]:QCUM[q + 1], :].opt()],
                    outs=[tfulls[L][q][:].opt()],
                )

            def agg(L, s, ag_hooks=None):
                stile0 = sup_tile0[s]
                stiles = sup_tile0[s + 1] - stile0
                smms = mm_of_sup[s]
                nmm_s = len(smms)
                mm0 = sum(len(mm_of_sup[ss]) for ss in range(s))
                gs = gpool.tile([128, stiles, 128], dt.bfloat16,
                                name=f"gs{L}_{s}", tag="gs",
                                padded_shape=[128, max_sup_tiles, 128])
                for (j, off, n, cs) in calls_by_sup.get(s, []):
                    if ag_hooks and j in ag_hooks:
                        ag_hooks.pop(j)()
                    rel = off // 128 - stile0
                    nc.gpsimd.dma_gather(
                        gs[:, rel:rel + (n + 127) // 128, :],
                        tfulls[L][j][:],
                        sidx_s[:, off // 16:(off + n) // 16],
                        n, n, 128,
                        queue_num=next_queue(),
                    )
                oh = ohpool.tile([128, nmm_s * 128], dt.bfloat16,
                                 name=f"oh{L}", tag="oh",
                                 padded_shape=[128, max_sup_mm * 128])
                q = 0
                while q < nmm_s:
                    nb = min(ohchunk, nmm_s - q)
                    dsl = dcol_s[:, mm0 + q:mm0 + q + nb]
                    bcast = AP(dsl.tensor, dsl.offset,
                               list(dsl.ap) + [[0, 128]])
                    nc.vector.tensor_tensor(
                        out=oh[:, q * 128:(q + nb) * 128],
                        in0=iota_s[:, 0:nb * 128],
                        in1=bcast,
                        op=mybir.AluOpType.is_equal)
                    q += nb
                # per-window PSUM accumulation over this super's matmul list
                aps_of_w = {}
                done_of_w = {}
                for mi, (w, j, t) in enumerate(smms):
                    if w not in aps_of_w:
                        aps_of_w[w] = ppool.tile([128, 128], dt.float32,
                                                 name=f"ap{L}", tag="pp")
                        done_of_w[w] = 0
                    aps = aps_of_w[w]
                    k = done_of_w[w]
                    ohsl = oh[:, mi * 128:(mi + 1) * 128]
                    first, last = k == 0, k == int(nmm_w[w]) - 1
                    if L < 2:
                        nc.tensor.matmul(aps[:], lhsT=gs[:, t - stile0, :],
                                         rhs=ohsl, start=first, stop=last)
                    else:
                        nc.tensor.matmul(aps[:, 0:D_OUT], lhsT=ohsl,
                                         rhs=gs[:, t - stile0, 0:D_OUT],
                                         start=first, stop=last)
                    done_of_w[w] = k + 1
                    if not last:
                        continue
                    if L < 2:
                        nc.scalar.activation(
                            h_s[:, w * 128:(w + 1) * 128], aps[:],
                            mybir.ActivationFunctionType.Relu)
                    else:
                        wsz = min(128, NP - w * 128)
                        ob = wpool.tile([128, D_OUT], dt.float32, name="ob",
                                        tag="ob")
                        nc.vector.tensor_scalar(
                            out=ob[:], in0=aps[:, 0:D_OUT],
                            scalar1=nd_s[:, w:w + 1], scalar2=None,
                            op0=mybir.AluOpType.mult)
                        nc.sync.dma_start(
                            out_t.ap()[w * 128:w * 128 + wsz, :], ob[:wsz, :])

            # ---- L0 projection (staged featT slices) + quarter-AGs ----
            agq = 0
            for w0 in range(0, NWIN, FTW):
                wn = min(FTW, NWIN - w0)
                fta = ftpool.tile([128, wn * 128], dt.bfloat16, name="fta",
                                  tag="fta", padded_shape=[128, FTW * 128])
                ftb = ftpool.tile([128, wn * 128], dt.bfloat16, name="ftb",
                                  tag="ftb", padded_shape=[128, FTW * 128])
                nc.sync.dma_start(
                    fta[:], featT_t.ap()[0, :, w0 * 128:(w0 + wn) * 128])
                nc.sync.dma_start(
                    ftb[:], featT_t.ap()[1, :, w0 * 128:(w0 + wn) * 128])
                for g0 in range(0, wn, 4):
                    gn = min(4, wn - g0)
                    proj4(0, w0 + g0, gn, fta, ftb, g0)
                    while agq < 1 and w0 + g0 + gn - 1 >= AGWIN[agq]:
                        ag(0, agq)
                        agq += 1

            # ---- pipelined layers ----
            agsup = [min(nsup - 1, (AGWIN[q] // SUPW) + 2) for q in range(NCHUNK)]
            for L in range(3):
                hooks = ({q: (lambda qq=q: ag(0, qq)) for q in (1, 2, 3)}
                         if L == 0 else None)
                nagq = 0
                for s in range(nsup):
                    agg(L, s, ag_hooks=hooks if s == 0 else None)
                    if L < 2:
                        proj4(L + 1, sup_ws[s][0], len(sup_ws[s]))
                        while nagq < NCHUNK and s >= agsup[nagq]:
                            ag(L + 1, nagq)
                            nagq += 1

    nc.compile()
    return nc


def _in_map(consts, percore, c):
    return {
        "featT": percore["featT"][c],
        "w1": consts["w1"], "w2": consts["w2"], "w3p": consts["w3p"],
        "nsp": percore["nsp"][c], "nsdp": percore["nsdp"][c],
        "ndp": percore["ndp"][c],
        "sidx": percore["sidx"][c], "dcol": percore["dcol"][c],
        "iota": consts["iota"],
    }


def kernel(feat, W1, b1, W2, b2, W3, b3, src, dst):
    sched, consts, percore = _host_prep(feat, W1, b1, W2, b2, W3, b3, src, dst)
    nc = _build(sched)
    in_maps = [_in_map(consts, percore, c) for c in range(NCORE)]
    res = run_bass_kernel_spmd(nc, in_maps, core_ids=list(range(NCORE)))
    out = np.concatenate([res.results[c]["out"][:NP] for c in range(NCORE)],
                         axis=0)
    return np.ascontiguousarray(out.astype(np.float32))


# revision 21
# speedup vs baseline: 1.2108x; 1.0160x over previous
"""3-layer GCN (DGL GraphConv, norm='both') on 8 Trainium2 NeuronCores.

v4: descriptor-minimized, software-pipelined SPMD single-NEFF design.
  - Nodes partitioned contiguously: core c owns rows [c*12500, (c+1)*12500).
  - Per layer: project own nodes on PE (bf16) -> [12500,128] bf16 shard;
    AllGather split into FOUR quarter-collectives that fire as soon as their
    projection windows complete (table chunk q = concat over cores of local
    rows [q*3125,(q+1)*3125), keeping int16 gather indices valid).
  - Per-edge SWDGE dma_gather fetches source rows; calls round-robin over 4
    SWDGE queues so all four Q7 core pairs generate descriptors in parallel.
  - Gather stream layout: cells (dst-window, src-chunk) sized to the
    cross-core max count and concatenated per (super-group, chunk) WITHOUT
    per-cell rounding (only the group tail pads to a tile). Tiles may span
    window boundaries; each (window, chunk, tile) matmul gets its own
    one-hot column block with -1 (no match) marking rows of other windows.
    This cuts gathered rows ~12% vs per-cell 128-rounding.
  - Segment-sum by dst via one-hot matmul accumulation in PSUM over 128-dst
    windows; one-hot built in wide DVE is_equal instrs.
  - Next-layer projection windows are emitted right after each super-group's
    aggregation, keeping PE/DMA/Q7 busy across layer boundaries.
  - Layers 1-2 keep h transposed ([feat x nodes]); the in-degree norm is
    deferred into the next projection's per-row scale (zero bias asserted).
  - Host (numpy) does index-only prep (degrees, bucketing, sorting, padding
    to a core-uniform static schedule - SPMD needs identical instruction
    streams on all 8 cores).
"""

import numpy as np
import ml_dtypes

import concourse.bacc as bacc
import concourse.bass as bass
import concourse.mybir as mybir
import concourse.tile as tile
from concourse.bass import AP
from concourse.bass_utils import run_bass_kernel_spmd

BF16 = ml_dtypes.bfloat16
F32 = np.float32

N_NODES = 100000
D_IN, D_H1, D_H2, D_OUT = 256, 128, 128, 64
NCORE = 8
NP = N_NODES // NCORE        # 12500 nodes per core
NWIN = (NP + 127) // 128     # 98 windows (last holds 84)
NPPAD = NWIN * 128           # 12544
NCHUNK = 4
# uneven quarters: the LAST quarter-AG gates each layer boundary (it needs
# the final projection window), so keep it small to shrink that exposed
# latency. max sloc = 8*3584-1 = 28671 stays within int16.
QROWS_L = [4032, 4032, 3072, NP - 11136]         # per-quarter rows per core
QCUM = [0]
for _q in QROWS_L:
    QCUM.append(QCUM[-1] + _q)
CHUNK_L = [q * NCORE for q in QROWS_L]           # gather chunk sizes
GCALL = 1024                 # idxs per dma_gather call
SUPW = 4                     # windows per gather super-group
NQUEUE = 4                   # SWDGE queues (Q7 core pairs)
FTW = 8                      # windows per staged featT slice (L0)
# last proj window needed by quarter-AG q
AGWIN = [((QCUM[q + 1] + 127) // 128) - 1 for q in range(NCHUNK)]


def _host_prep(feat, W1, b1, W2, b2, W3, b3, src, dst):
    src = np.asarray(src).astype(np.int64)
    dst = np.asarray(dst).astype(np.int64)
    for b in (b1, b2, b3):
        assert np.max(np.abs(np.asarray(b))) == 0.0, \
            "nonzero bias needs the undeferred-nd path"

    deg_out = np.bincount(src, minlength=N_NODES).astype(F32)
    deg_in = np.bincount(dst, minlength=N_NODES).astype(F32)
    ns = 1.0 / np.sqrt(np.maximum(deg_out, 1.0))
    nd = 1.0 / np.sqrt(np.maximum(deg_in, 1.0))
    nsd = ns * nd

    core = dst // NP
    dloc = dst % NP
    win = dloc // 128
    c_src = src // NP
    r_src = src % NP
    chunk = np.searchsorted(np.array(QCUM[1:]), r_src, side="right")
    qr = np.array([QROWS_L[q] for q in chunk])
    q0 = np.array([QCUM[q] for q in chunk])
    sloc = (c_src * qr + (r_src - q0)).astype(np.int16)
    dcol = (dloc % 128).astype(np.int32)

    ncell = NCORE * NWIN * NCHUNK
    cell = ((core * NWIN + win) * NCHUNK + chunk).astype(np.int64)
    order = np.argsort(cell * 32768 + sloc, kind="stable")
    counts = np.bincount(cell, minlength=ncell).reshape(NCORE, NWIN, NCHUNK)
    m_wj = counts.max(axis=0).astype(np.int64)      # uniform cell sizes

    starts = np.zeros(ncell + 1, np.int64)
    np.cumsum(np.bincount(cell, minlength=ncell), out=starts[1:])

    nsup = (NWIN + SUPW - 1) // SUPW
    sup_ws = [list(range(s * SUPW, min((s + 1) * SUPW, NWIN)))
              for s in range(nsup)]

    # slot layout: per (super, chunk) group: cells concatenated (no per-cell
    # rounding), group padded to a tile boundary.
    A_wj = np.zeros((NWIN, NCHUNK), np.int64)   # absolute slot offset of cell
    gtile0 = {}                                  # (s, j) -> first abs tile
    gtiles = {}                                  # (s, j) -> tiles in group
    sup_tile0 = []                               # first abs tile of super
    p = 0                                        # in slots (always 128-aligned
    #                                              at group boundaries)
    for s in range(nsup):
        sup_tile0.append(p // 128)
        for j in range(NCHUNK):
            gtile0[(s, j)] = p // 128
            g0 = p
            for w in sup_ws[s]:
                A_wj[w, j] = p
                p += int(m_wj[w, j])
            p = ((p + 127) // 128) * 128
            gtiles[(s, j)] = (p - g0) // 128
    NT2 = p // 128                               # total stream tiles
    sup_tile0.append(NT2)

    # matmul list per super: (w, j, abs_tile) in emission order; per-window
    # start/stop counts
    mm_of_sup = []                               # list per super of (w,j,t)
    nmm_w = np.zeros(NWIN, np.int64)
    for s in range(nsup):
        lst = []
        for w in sup_ws[s]:
            for j in range(NCHUNK):
                if m_wj[w, j] == 0:
                    continue
                a, m = int(A_wj[w, j]), int(m_wj[w, j])
                for t in range(a // 128, (a + m + 127) // 128):
                    lst.append((w, j, t))
                    nmm_w[w] += 1
        mm_of_sup.append(lst)
    NMM = int(sum(len(x) for x in mm_of_sup))
    max_sup_mm = max(len(x) for x in mm_of_sup)
    max_sup_tiles = max(sup_tile0[s + 1] - sup_tile0[s] for s in range(nsup))

    # per-core slot data
    sidx = np.full((NCORE, NT2 * 128), -1, np.int16)
    dcol_slot = np.full((NCORE, NT2 * 128), -1, np.int64)
    for c in range(NCORE):
        for w in range(NWIN):
            for j in range(NCHUNK):
                cid = (c * NWIN + w) * NCHUNK + j
                e = order[starts[cid]:starts[cid + 1]]
                n = len(e)
                a = int(A_wj[w, j])
                sidx[c, a:a + n] = sloc[e]
                sidx[c, a + n:a + int(m_wj[w, j])] = 0   # interior pad
                dcol_slot[c, a:a + n] = dcol[e]
    # group tails keep idx=-1 (stripped at call end by the Q7)

    # per-matmul one-hot columns: [NMM, 128] values or -1
    dcol_mm = np.full((NCORE, NMM, 128), -1.0, F32)
    mm_idx = 0
    for s in range(nsup):
        for (w, j, t) in mm_of_sup[s]:
            a, m = int(A_wj[w, j]), int(m_wj[w, j])
            lo = max(a, t * 128)
            hi = min(a + m, (t + 1) * 128)
            for c in range(NCORE):
                seg = dcol_slot[c, lo:hi]
                dst_rows = np.arange(lo - t * 128, hi - t * 128)
                valid = seg >= 0
                dcol_mm[c, mm_idx, dst_rows[valid]] = seg[valid]
            mm_idx += 1
    assert mm_idx == NMM

    def idx_layout(a):      # [n] int16 -> [128, n//16]
        return np.tile(a.reshape(-1, 16).T, (8, 1))

    sidx_l = np.stack([idx_layout(sidx[c]) for c in range(NCORE)])
    dcol_l = np.ascontiguousarray(
        dcol_mm.transpose(0, 2, 1)).astype(BF16)    # [NCORE, 128, NMM]

    calls = []   # (chunk j, abs slot offset, n_idxs, super)
    for s in range(nsup):
        for j in range(NCHUNK):
            t0, nt = gtile0[(s, j)], gtiles[(s, j)]
            off = t0 * 128
            greal = int(sum(m_wj[w, j] for w in sup_ws[s]))   # real slots
            q = 0
            while q < greal:
                n = min(GCALL, ((greal - q + 15) // 16) * 16)
                calls.append((j, off + q, n, s))
                q += n

    feat = np.asarray(feat).astype(F32)
    featT = np.zeros((NCORE, 2, 128, NPPAD), BF16)
    nsp = np.zeros((NCORE, 128, NWIN), F32)
    nsdp = np.zeros((NCORE, 128, NWIN), F32)
    ndp = np.zeros((NCORE, 128, NWIN), F32)
    for c in range(NCORE):
        ft = feat[c * NP:(c + 1) * NP].astype(BF16).T   # [256, NP]
        featT[c, 0, :, :NP] = ft[0:128]
        featT[c, 1, :, :NP] = ft[128:256]
        for arr, dstp in ((ns, nsp), (nsd, nsdp), (nd, ndp)):
            v = np.zeros(NPPAD, F32)
            v[:NP] = arr[c * NP:(c + 1) * NP]
            dstp[c] = v.reshape(NWIN, 128).T

    ohchunk = min(32, max_sup_mm)
    consts = dict(
        w1=np.asarray(W1).astype(F32).astype(BF16),
        w2=np.asarray(W2).astype(F32).astype(BF16),
        w3p=np.pad(np.asarray(W3).astype(F32), ((0, 0), (0, 128 - D_OUT))).astype(BF16),
        iota=np.tile(np.arange(128, dtype=F32).astype(BF16)[None, :],
                     (128, ohchunk)),
    )
    sched = dict(NT2=NT2, NMM=NMM, calls=calls, mm_of_sup=mm_of_sup,
                 nmm_w=nmm_w, sup_tile0=sup_tile0, nsup=nsup, sup_ws=sup_ws,
                 max_sup_tiles=max_sup_tiles, max_sup_mm=max_sup_mm,
                 ohchunk=ohchunk)
    percore = dict(featT=featT, nsp=nsp, nsdp=nsdp, ndp=ndp,
                   sidx=sidx_l, dcol=dcol_l)
    return sched, consts, percore


def _build(sched):
    NT2 = sched["NT2"]; NMM = sched["NMM"]; calls = sched["calls"]
    mm_of_sup = sched["mm_of_sup"]; nmm_w = sched["nmm_w"]
    sup_tile0 = sched["sup_tile0"]; nsup = sched["nsup"]
    sup_ws = sched["sup_ws"]; max_sup_tiles = sched["max_sup_tiles"]
    max_sup_mm = sched["max_sup_mm"]; ohchunk = sched["ohchunk"]

    calls_by_sup = {}
    for c in calls:
        calls_by_sup.setdefault(c[3], []).append(c)
    # first matmul index (within super) per super, per window: for start/stop
    nc = bacc.Bacc("TRN2", target_bir_lowering=False, debug=False,
                   num_devices=NCORE, num_swdge_queues=NQUEUE)
    dt = mybir.dt

    featT_t = nc.dram_tensor("featT", [2, 128, NPPAD], dt.bfloat16,
                             kind="ExternalInput")
    w1_t = nc.dram_tensor("w1", [D_IN, D_H1], dt.bfloat16, kind="ExternalInput")
    w2_t = nc.dram_tensor("w2", [D_H1, D_H2], dt.bfloat16, kind="ExternalInput")
    w3_t = nc.dram_tensor("w3p", [D_H2, 128], dt.bfloat16, kind="ExternalInput")
    ns_t = nc.dram_tensor("nsp", [128, NWIN], dt.float32, kind="ExternalInput")
    nsd_t = nc.dram_tensor("nsdp", [128, NWIN], dt.float32, kind="ExternalInput")
    nd_t = nc.dram_tensor("ndp", [128, NWIN], dt.float32, kind="ExternalInput")
    sidx_t = nc.dram_tensor("sidx", [128, NT2 * 8], dt.int16, kind="ExternalInput")
    dcol_t = nc.dram_tensor("dcol", [128, NMM], dt.bfloat16, kind="ExternalInput")
    iota_t = nc.dram_tensor("iota", [128, ohchunk * 128], dt.bfloat16,
                            kind="ExternalInput")
    out_t = nc.dram_tensor("out", [NP, D_OUT], dt.float32, kind="ExternalOutput")

    qcount = [0]

    def next_queue():
        q = qcount[0] % NQUEUE
        qcount[0] += 1
        return q

    with tile.TileContext(nc) as tc:
        with (
            tc.tile_pool(name="const", bufs=1) as cpool,
            tc.tile_pool(name="hbuf", bufs=1) as hpool,
            tc.tile_pool(name="gb", bufs=3) as gpool,
            tc.tile_pool(name="ft", bufs=2) as ftpool,
            tc.tile_pool(name="work", bufs=3) as wpool,
            tc.tile_pool(name="oh", bufs=3) as ohpool,
            tc.tile_pool(name="ps", bufs=6, space="PSUM") as ppool,
            tc.tile_pool(name="pj", bufs=2, space="PSUM") as pjpool,
            tc.tile_pool(name="dram", bufs=1, space="DRAM") as dpool,
        ):
            w1a_s = cpool.tile([128, D_H1], dt.bfloat16)
            w1b_s = cpool.tile([128, D_H1], dt.bfloat16)
            w2_s = cpool.tile([D_H1, D_H2], dt.bfloat16)
            w3_s = cpool.tile([D_H2, 128], dt.bfloat16)
            ns_s = cpool.tile([128, NWIN], dt.float32)
            nsd_s = cpool.tile([128, NWIN], dt.float32)
            nd_s = cpool.tile([128, NWIN], dt.float32)
            sidx_s = cpool.tile([128, NT2 * 8], dt.int16)
            dcol_s = cpool.tile([128, NMM], dt.bfloat16)
            iota_s = cpool.tile([128, ohchunk * 128], dt.bfloat16)

            nc.sync.dma_start(w1a_s[:], w1_t.ap()[0:128, :])
            nc.sync.dma_start(w1b_s[:], w1_t.ap()[128:256, :])
            nc.sync.dma_start(w2_s[:], w2_t.ap())
            nc.sync.dma_start(w3_s[:], w3_t.ap())
            nc.sync.dma_start(ns_s[:], ns_t.ap())
            nc.sync.dma_start(nsd_s[:], nsd_t.ap())
            nc.sync.dma_start(nd_s[:], nd_t.ap())
            nc.sync.dma_start(sidx_s[:], sidx_t.ap())
            nc.sync.dma_start(dcol_s[:], dcol_t.ap())
            nc.sync.dma_start(iota_s[:], iota_t.ap())

            h_s = hpool.tile([128, NWIN * 128], dt.bfloat16)   # hT (feat x nodes)

            tins = [dpool.tile([NP, 128], dt.bfloat16, name=f"tin{L}")
                    for L in range(3)]
            tfulls = [[dpool.tile([CHUNK_L[q], 128], dt.bfloat16,
                                  name=f"tfull{L}_{q}", addr_space="Shared")
                       for q in range(NCHUNK)] for L in range(3)]

            # zero the two gather buffers once: stale bytes multiply with
            # one-hot zeros, so they must be finite (not NaN bit patterns)
            for _ in range(3):
                gz = gpool.tile([128, max_sup_tiles, 128], dt.bfloat16,
                                name="gsz", tag="gs",
                                padded_shape=[128, max_sup_tiles, 128])
                nc.vector.memset(gz[:], 0.0)

            def proj4(L, w0, wn, fta=None, ftb=None, k0=0):
                # wn (<=4) projection windows into one PSUM bank, one wide
                # scale instr, per-window tin writes
                ppj = pjpool.tile([128, wn * 128], dt.float32, name=f"pj{L}",
                                  tag="pj", padded_shape=[128, 512])
                for k in range(wn):
                    w = w0 + k
                    osl = ppj[:, k * 128:(k + 1) * 128]
                    if L == 0:
                        nc.tensor.matmul(osl,
                                         lhsT=fta[:, (k0 + k) * 128:(k0 + k + 1) * 128],
                                         rhs=w1a_s[:], start=True, stop=False)
                        nc.tensor.matmul(osl,
                                         lhsT=ftb[:, (k0 + k) * 128:(k0 + k + 1) * 128],
                                         rhs=w1b_s[:], start=False, stop=True)
                    else:
                        rhs = w2_s if L == 1 else w3_s
                        nc.tensor.matmul(osl,
                                         lhsT=h_s[:, w * 128:(w + 1) * 128],
                                         rhs=rhs[:], start=True, stop=True)
                scal = ns_s if L == 0 else nsd_s
                ssl = scal[:, w0:w0 + wn]
                sbc = AP(ssl.tensor, ssl.offset, list(ssl.ap) + [[0, 128]])
                pbf = wpool.tile([128, wn * 128], dt.bfloat16, name="pbf",
                                 tag="pbf", padded_shape=[128, 512])
                nc.vector.tensor_tensor(out=pbf[:, 0:wn * 128],
                                        in0=ppj[:, 0:wn * 128], in1=sbc,
                                        op=mybir.AluOpType.mult)
                for k in range(wn):
                    w = w0 + k
                    wsz = min(128, NP - w * 128)
                    nc.sync.dma_start(tins[L][w * 128:w * 128 + wsz, :],
                                      pbf[:wsz, k * 128:(k + 1) * 128])

            def ag(L, q):
                nc.gpsimd.collective_compute(
                    "AllGather", mybir.AluOpType.bypass,
                    replica_groups=[list(range(NCORE))],
                    ins=[tins[L][QCUM[q]:QCUM[q + 1], :].opt()],
                    outs=[tfulls[L][q][:].opt()],
                )

            def agg(L, s, ag_hooks=None):
                stile0 = sup_tile0[s]
                stiles = sup_tile0[s + 1] - stile0
                smms = mm_of_sup[s]
                nmm_s = len(smms)
                mm0 = sum(len(mm_of_sup[ss]) for ss in range(s))
                gs = gpool.tile([128, stiles, 128], dt.bfloat16,
                                name=f"gs{L}_{s}", tag="gs",
                                padded_shape=[128, max_sup_tiles, 128])
                for (j, off, n, cs) in calls_by_sup.get(s, []):
                    if ag_hooks and j in ag_hooks:
                        ag_hooks.pop(j)()
                    rel = off // 128 - stile0
                    nc.gpsimd.dma_gather(
                        gs[:, rel:rel + (n + 127) // 128, :],
                        tfulls[L][j][:],
                        sidx_s[:, off // 16:(off + n) // 16],
                        n, n, 128,
                        queue_num=next_queue(),
                    )
                oh = ohpool.tile([128, nmm_s * 128], dt.bfloat16,
                                 name=f"oh{L}", tag="oh",
                                 padded_shape=[128, max_sup_mm * 128])
                q = 0
                while q < nmm_s:
                    nb = min(ohchunk, nmm_s - q)
                    dsl = dcol_s[:, mm0 + q:mm0 + q + nb]
                    bcast = AP(dsl.tensor, dsl.offset,
                               list(dsl.ap) + [[0, 128]])
                    nc.vector.tensor_tensor(
                        out=oh[:, q * 128:(q + nb) * 128],
                        in0=iota_s[:, 0:nb * 128],
                        in1=bcast,
                        op=mybir.AluOpType.is_equal)
                    q += nb
                # per-window PSUM accumulation over this super's matmul list
                aps_of_w = {}
                done_of_w = {}
                for mi, (w, j, t) in enumerate(smms):
                    if w not in aps_of_w:
                        aps_of_w[w] = ppool.tile([128, 128], dt.float32,
                                                 name=f"ap{L}", tag="pp")
                        done_of_w[w] = 0
                    aps = aps_of_w[w]
                    k = done_of_w[w]
                    ohsl = oh[:, mi * 128:(mi + 1) * 128]
                    first, last = k == 0, k == int(nmm_w[w]) - 1
                    if L < 2:
                        nc.tensor.matmul(aps[:], lhsT=gs[:, t - stile0, :],
                                         rhs=ohsl, start=first, stop=last)
                    else:
                        nc.tensor.matmul(aps[:, 0:D_OUT], lhsT=ohsl,
                                         rhs=gs[:, t - stile0, 0:D_OUT],
                                         start=first, stop=last)
                    done_of_w[w] = k + 1
                    if not last:
                        continue
                    if L < 2:
                        nc.scalar.activation(
                            h_s[:, w * 128:(w + 1) * 128], aps[:],
                            mybir.ActivationFunctionType.Relu)
                    else:
                        wsz = min(128, NP - w * 128)
                        ob = wpool.tile([128, D_OUT], dt.float32, name="ob",
                                        tag="ob")
                        nc.vector.tensor_scalar(
                            out=ob[:], in0=aps[:, 0:D_OUT],
                            scalar1=nd_s[:, w:w + 1], scalar2=None,
                            op0=mybir.AluOpType.mult)
                        nc.sync.dma_start(
                            out_t.ap()[w * 128:w * 128 + wsz, :], ob[:wsz, :])

            # ---- L0 projection (staged featT slices) + quarter-AGs ----
            agq = 0
            for w0 in range(0, NWIN, FTW):
                wn = min(FTW, NWIN - w0)
                fta = ftpool.tile([128, wn * 128], dt.bfloat16, name="fta",
                                  tag="fta", padded_shape=[128, FTW * 128])
                ftb = ftpool.tile([128, wn * 128], dt.bfloat16, name="ftb",
                                  tag="ftb", padded_shape=[128, FTW * 128])
                nc.sync.dma_start(
                    fta[:], featT_t.ap()[0, :, w0 * 128:(w0 + wn) * 128])
                nc.sync.dma_start(
                    ftb[:], featT_t.ap()[1, :, w0 * 128:(w0 + wn) * 128])
                for g0 in range(0, wn, 4):
                    gn = min(4, wn - g0)
                    proj4(0, w0 + g0, gn, fta, ftb, g0)
                    while agq < 1 and w0 + g0 + gn - 1 >= AGWIN[agq]:
                        ag(0, agq)
                        agq += 1

            # ---- pipelined layers ----
            agsup = [min(nsup - 1, (AGWIN[q] // SUPW) + 2) for q in range(NCHUNK)]
            for L in range(3):
                hooks = ({q: (lambda qq=q: ag(0, qq)) for q in (1, 2, 3)}
                         if L == 0 else None)
                nagq = 0
                for s in range(nsup):
                    agg(L, s, ag_hooks=hooks if s == 0 else None)
                    if L < 2:
                        proj4(L + 1, sup_ws[s][0], len(sup_ws[s]))
                        while nagq < NCHUNK and s >= agsup[nagq]:
                            ag(L + 1, nagq)
                            nagq += 1

    nc.compile()
    return nc


def _in_map(consts, percore, c):
    return {
        "featT": percore["featT"][c],
        "w1": consts["w1"], "w2": consts["w2"], "w3p": consts["w3p"],
        "nsp": percore["nsp"][c], "nsdp": percore["nsdp"][c],
        "ndp": percore["ndp"][c],
        "sidx": percore["sidx"][c], "dcol": percore["dcol"][c],
        "iota": consts["iota"],
    }


def kernel(feat, W1, b1, W2, b2, W3, b3, src, dst):
    sched, consts, percore = _host_prep(feat, W1, b1, W2, b2, W3, b3, src, dst)
    nc = _build(sched)
    in_maps = [_in_map(consts, percore, c) for c in range(NCORE)]
    res = run_bass_kernel_spmd(nc, in_maps, core_ids=list(range(NCORE)))
    out = np.concatenate([res.results[c]["out"][:NP] for c in range(NCORE)],
                         axis=0)
    return np.ascontiguousarray(out.astype(np.float32))


# revision 22
# speedup vs baseline: 1.2139x; 1.0026x over previous
"""3-layer GCN (DGL GraphConv, norm='both') on 8 Trainium2 NeuronCores.

v4: descriptor-minimized, software-pipelined SPMD single-NEFF design.
  - Nodes partitioned contiguously: core c owns rows [c*12500, (c+1)*12500).
  - Per layer: project own nodes on PE (bf16) -> [12500,128] bf16 shard;
    AllGather split into FOUR quarter-collectives that fire as soon as their
    projection windows complete (table chunk q = concat over cores of local
    rows [q*3125,(q+1)*3125), keeping int16 gather indices valid).
  - Per-edge SWDGE dma_gather fetches source rows; calls round-robin over 4
    SWDGE queues so all four Q7 core pairs generate descriptors in parallel.
  - Gather stream layout: cells (dst-window, src-chunk) sized to the
    cross-core max count and concatenated per (super-group, chunk) WITHOUT
    per-cell rounding (only the group tail pads to a tile). Tiles may span
    window boundaries; each (window, chunk, tile) matmul gets its own
    one-hot column block with -1 (no match) marking rows of other windows.
    This cuts gathered rows ~12% vs per-cell 128-rounding.
  - Segment-sum by dst via one-hot matmul accumulation in PSUM over 128-dst
    windows; one-hot built in wide DVE is_equal instrs.
  - Next-layer projection windows are emitted right after each super-group's
    aggregation, keeping PE/DMA/Q7 busy across layer boundaries.
  - Layers 1-2 keep h transposed ([feat x nodes]); the in-degree norm is
    deferred into the next projection's per-row scale (zero bias asserted).
  - Host (numpy) does index-only prep (degrees, bucketing, sorting, padding
    to a core-uniform static schedule - SPMD needs identical instruction
    streams on all 8 cores).
"""

import numpy as np
import ml_dtypes

import concourse.bacc as bacc
import concourse.bass as bass
import concourse.mybir as mybir
import concourse.tile as tile
from concourse.bass import AP
from concourse.bass_utils import run_bass_kernel_spmd

BF16 = ml_dtypes.bfloat16
F32 = np.float32

N_NODES = 100000
D_IN, D_H1, D_H2, D_OUT = 256, 128, 128, 64
NCORE = 8
NP = N_NODES // NCORE        # 12500 nodes per core
NWIN = (NP + 127) // 128     # 98 windows (last holds 84)
NPPAD = NWIN * 128           # 12544
NCHUNK = 4
# uneven quarters: the LAST quarter-AG gates each layer boundary (it needs
# the final projection window), so keep it small to shrink that exposed
# latency. max sloc = 8*3584-1 = 28671 stays within int16.
QROWS_L = [4064, 4064, 3328, NP - 11456]         # per-quarter rows per core
QCUM = [0]
for _q in QROWS_L:
    QCUM.append(QCUM[-1] + _q)
CHUNK_L = [q * NCORE for q in QROWS_L]           # gather chunk sizes
GCALL = 1024                 # idxs per dma_gather call
SUPW = 4                     # windows per gather super-group
NQUEUE = 4                   # SWDGE queues (Q7 core pairs)
FTW = 8                      # windows per staged featT slice (L0)
# last proj window needed by quarter-AG q
AGWIN = [((QCUM[q + 1] + 127) // 128) - 1 for q in range(NCHUNK)]


def _host_prep(feat, W1, b1, W2, b2, W3, b3, src, dst):
    src = np.asarray(src).astype(np.int64)
    dst = np.asarray(dst).astype(np.int64)
    for b in (b1, b2, b3):
        assert np.max(np.abs(np.asarray(b))) == 0.0, \
            "nonzero bias needs the undeferred-nd path"

    deg_out = np.bincount(src, minlength=N_NODES).astype(F32)
    deg_in = np.bincount(dst, minlength=N_NODES).astype(F32)
    ns = 1.0 / np.sqrt(np.maximum(deg_out, 1.0))
    nd = 1.0 / np.sqrt(np.maximum(deg_in, 1.0))
    nsd = ns * nd

    core = dst // NP
    dloc = dst % NP
    win = dloc // 128
    c_src = src // NP
    r_src = src % NP
    chunk = np.searchsorted(np.array(QCUM[1:]), r_src, side="right")
    qr = np.array([QROWS_L[q] for q in chunk])
    q0 = np.array([QCUM[q] for q in chunk])
    sloc = (c_src * qr + (r_src - q0)).astype(np.int16)
    dcol = (dloc % 128).astype(np.int32)

    ncell = NCORE * NWIN * NCHUNK
    cell = ((core * NWIN + win) * NCHUNK + chunk).astype(np.int64)
    order = np.argsort(cell * 32768 + sloc, kind="stable")
    counts = np.bincount(cell, minlength=ncell).reshape(NCORE, NWIN, NCHUNK)
    m_wj = counts.max(axis=0).astype(np.int64)      # uniform cell sizes

    starts = np.zeros(ncell + 1, np.int64)
    np.cumsum(np.bincount(cell, minlength=ncell), out=starts[1:])

    nsup = (NWIN + SUPW - 1) // SUPW
    sup_ws = [list(range(s * SUPW, min((s + 1) * SUPW, NWIN)))
              for s in range(nsup)]

    # slot layout: per (super, chunk) group: cells concatenated (no per-cell
    # rounding), group padded to a tile boundary.
    A_wj = np.zeros((NWIN, NCHUNK), np.int64)   # absolute slot offset of cell
    gtile0 = {}                                  # (s, j) -> first abs tile
    gtiles = {}                                  # (s, j) -> tiles in group
    sup_tile0 = []                               # first abs tile of super
    p = 0                                        # in slots (always 128-aligned
    #                                              at group boundaries)
    for s in range(nsup):
        sup_tile0.append(p // 128)
        for j in range(NCHUNK):
            gtile0[(s, j)] = p // 128
            g0 = p
            for w in sup_ws[s]:
                A_wj[w, j] = p
                p += int(m_wj[w, j])
            p = ((p + 127) // 128) * 128
            gtiles[(s, j)] = (p - g0) // 128
    NT2 = p // 128                               # total stream tiles
    sup_tile0.append(NT2)

    # matmul list per super: (w, j, abs_tile) in emission order; per-window
    # start/stop counts
    mm_of_sup = []                               # list per super of (w,j,t)
    nmm_w = np.zeros(NWIN, np.int64)
    for s in range(nsup):
        lst = []
        for w in sup_ws[s]:
            for j in range(NCHUNK):
                if m_wj[w, j] == 0:
                    continue
                a, m = int(A_wj[w, j]), int(m_wj[w, j])
                for t in range(a // 128, (a + m + 127) // 128):
                    lst.append((w, j, t))
                    nmm_w[w] += 1
        mm_of_sup.append(lst)
    NMM = int(sum(len(x) for x in mm_of_sup))
    max_sup_mm = max(len(x) for x in mm_of_sup)
    max_sup_tiles = max(sup_tile0[s + 1] - sup_tile0[s] for s in range(nsup))

    # per-core slot data
    sidx = np.full((NCORE, NT2 * 128), -1, np.int16)
    dcol_slot = np.full((NCORE, NT2 * 128), -1, np.int64)
    for c in range(NCORE):
        for w in range(NWIN):
            for j in range(NCHUNK):
                cid = (c * NWIN + w) * NCHUNK + j
                e = order[starts[cid]:starts[cid + 1]]
                n = len(e)
                a = int(A_wj[w, j])
                sidx[c, a:a + n] = sloc[e]
                sidx[c, a + n:a + int(m_wj[w, j])] = 0   # interior pad
                dcol_slot[c, a:a + n] = dcol[e]
    # group tails keep idx=-1 (stripped at call end by the Q7)

    # per-matmul one-hot columns: [NMM, 128] values or -1
    dcol_mm = np.full((NCORE, NMM, 128), -1.0, F32)
    mm_idx = 0
    for s in range(nsup):
        for (w, j, t) in mm_of_sup[s]:
            a, m = int(A_wj[w, j]), int(m_wj[w, j])
            lo = max(a, t * 128)
            hi = min(a + m, (t + 1) * 128)
            for c in range(NCORE):
                seg = dcol_slot[c, lo:hi]
                dst_rows = np.arange(lo - t * 128, hi - t * 128)
                valid = seg >= 0
                dcol_mm[c, mm_idx, dst_rows[valid]] = seg[valid]
            mm_idx += 1
    assert mm_idx == NMM

    def idx_layout(a):      # [n] int16 -> [128, n//16]
        return np.tile(a.reshape(-1, 16).T, (8, 1))

    sidx_l = np.stack([idx_layout(sidx[c]) for c in range(NCORE)])
    dcol_l = np.ascontiguousarray(
        dcol_mm.transpose(0, 2, 1)).astype(BF16)    # [NCORE, 128, NMM]

    calls = []   # (chunk j, abs slot offset, n_idxs, super)
    for s in range(nsup):
        for j in range(NCHUNK):
            t0, nt = gtile0[(s, j)], gtiles[(s, j)]
            off = t0 * 128
            greal = int(sum(m_wj[w, j] for w in sup_ws[s]))   # real slots
            q = 0
            while q < greal:
                n = min(GCALL, ((greal - q + 15) // 16) * 16)
                calls.append((j, off + q, n, s))
                q += n

    feat = np.asarray(feat).astype(F32)
    featT = np.zeros((NCORE, 2, 128, NPPAD), BF16)
    nsp = np.zeros((NCORE, 128, NWIN), F32)
    nsdp = np.zeros((NCORE, 128, NWIN), F32)
    ndp = np.zeros((NCORE, 128, NWIN), F32)
    for c in range(NCORE):
        ft = feat[c * NP:(c + 1) * NP].astype(BF16).T   # [256, NP]
        featT[c, 0, :, :NP] = ft[0:128]
        featT[c, 1, :, :NP] = ft[128:256]
        for arr, dstp in ((ns, nsp), (nsd, nsdp), (nd, ndp)):
            v = np.zeros(NPPAD, F32)
            v[:NP] = arr[c * NP:(c + 1) * NP]
            dstp[c] = v.reshape(NWIN, 128).T

    ohchunk = min(32, max_sup_mm)
    consts = dict(
        w1=np.asarray(W1).astype(F32).astype(BF16),
        w2=np.asarray(W2).astype(F32).astype(BF16),
        w3p=np.pad(np.asarray(W3).astype(F32), ((0, 0), (0, 128 - D_OUT))).astype(BF16),
        iota=np.tile(np.arange(128, dtype=F32).astype(BF16)[None, :],
                     (128, ohchunk)),
    )
    sched = dict(NT2=NT2, NMM=NMM, calls=calls, mm_of_sup=mm_of_sup,
                 nmm_w=nmm_w, sup_tile0=sup_tile0, nsup=nsup, sup_ws=sup_ws,
                 max_sup_tiles=max_sup_tiles, max_sup_mm=max_sup_mm,
                 ohchunk=ohchunk)
    percore = dict(featT=featT, nsp=nsp, nsdp=nsdp, ndp=ndp,
                   sidx=sidx_l, dcol=dcol_l)
    return sched, consts, percore


def _build(sched):
    NT2 = sched["NT2"]; NMM = sched["NMM"]; calls = sched["calls"]
    mm_of_sup = sched["mm_of_sup"]; nmm_w = sched["nmm_w"]
    sup_tile0 = sched["sup_tile0"]; nsup = sched["nsup"]
    sup_ws = sched["sup_ws"]; max_sup_tiles = sched["max_sup_tiles"]
    max_sup_mm = sched["max_sup_mm"]; ohchunk = sched["ohchunk"]

    calls_by_sup = {}
    for c in calls:
        calls_by_sup.setdefault(c[3], []).append(c)
    # first matmul index (within super) per super, per window: for start/stop
    nc = bacc.Bacc("TRN2", target_bir_lowering=False, debug=False,
                   num_devices=NCORE, num_swdge_queues=NQUEUE)
    dt = mybir.dt

    featT_t = nc.dram_tensor("featT", [2, 128, NPPAD], dt.bfloat16,
                             kind="ExternalInput")
    w1_t = nc.dram_tensor("w1", [D_IN, D_H1], dt.bfloat16, kind="ExternalInput")
    w2_t = nc.dram_tensor("w2", [D_H1, D_H2], dt.bfloat16, kind="ExternalInput")
    w3_t = nc.dram_tensor("w3p", [D_H2, 128], dt.bfloat16, kind="ExternalInput")
    ns_t = nc.dram_tensor("nsp", [128, NWIN], dt.float32, kind="ExternalInput")
    nsd_t = nc.dram_tensor("nsdp", [128, NWIN], dt.float32, kind="ExternalInput")
    nd_t = nc.dram_tensor("ndp", [128, NWIN], dt.float32, kind="ExternalInput")
    sidx_t = nc.dram_tensor("sidx", [128, NT2 * 8], dt.int16, kind="ExternalInput")
    dcol_t = nc.dram_tensor("dcol", [128, NMM], dt.bfloat16, kind="ExternalInput")
    iota_t = nc.dram_tensor("iota", [128, ohchunk * 128], dt.bfloat16,
                            kind="ExternalInput")
    out_t = nc.dram_tensor("out", [NP, D_OUT], dt.float32, kind="ExternalOutput")

    qcount = [0]

    def next_queue():
        q = qcount[0] % NQUEUE
        qcount[0] += 1
        return q

    with tile.TileContext(nc) as tc:
        with (
            tc.tile_pool(name="const", bufs=1) as cpool,
            tc.tile_pool(name="hbuf", bufs=1) as hpool,
            tc.tile_pool(name="gb", bufs=3) as gpool,
            tc.tile_pool(name="ft", bufs=2) as ftpool,
            tc.tile_pool(name="work", bufs=3) as wpool,
            tc.tile_pool(name="oh", bufs=3) as ohpool,
            tc.tile_pool(name="ps", bufs=6, space="PSUM") as ppool,
            tc.tile_pool(name="pj", bufs=2, space="PSUM") as pjpool,
            tc.tile_pool(name="dram", bufs=1, space="DRAM") as dpool,
        ):
            w1a_s = cpool.tile([128, D_H1], dt.bfloat16)
            w1b_s = cpool.tile([128, D_H1], dt.bfloat16)
            w2_s = cpool.tile([D_H1, D_H2], dt.bfloat16)
            w3_s = cpool.tile([D_H2, 128], dt.bfloat16)
            ns_s = cpool.tile([128, NWIN], dt.float32)
            nsd_s = cpool.tile([128, NWIN], dt.float32)
            nd_s = cpool.tile([128, NWIN], dt.float32)
            sidx_s = cpool.tile([128, NT2 * 8], dt.int16)
            dcol_s = cpool.tile([128, NMM], dt.bfloat16)
            iota_s = cpool.tile([128, ohchunk * 128], dt.bfloat16)

            nc.sync.dma_start(w1a_s[:], w1_t.ap()[0:128, :])
            nc.sync.dma_start(w1b_s[:], w1_t.ap()[128:256, :])
            nc.sync.dma_start(w2_s[:], w2_t.ap())
            nc.sync.dma_start(w3_s[:], w3_t.ap())
            nc.sync.dma_start(ns_s[:], ns_t.ap())
            nc.sync.dma_start(nsd_s[:], nsd_t.ap())
            nc.sync.dma_start(nd_s[:], nd_t.ap())
            nc.sync.dma_start(sidx_s[:], sidx_t.ap())
            nc.sync.dma_start(dcol_s[:], dcol_t.ap())
            nc.sync.dma_start(iota_s[:], iota_t.ap())

            h_s = hpool.tile([128, NWIN * 128], dt.bfloat16)   # hT (feat x nodes)

            tins = [dpool.tile([NP, 128], dt.bfloat16, name=f"tin{L}")
                    for L in range(3)]
            tfulls = [[dpool.tile([CHUNK_L[q], 128], dt.bfloat16,
                                  name=f"tfull{L}_{q}", addr_space="Shared")
                       for q in range(NCHUNK)] for L in range(3)]

            # zero the two gather buffers once: stale bytes multiply with
            # one-hot zeros, so they must be finite (not NaN bit patterns)
            for _ in range(3):
                gz = gpool.tile([128, max_sup_tiles, 128], dt.bfloat16,
                                name="gsz", tag="gs",
                                padded_shape=[128, max_sup_tiles, 128])
                nc.vector.memset(gz[:], 0.0)

            def proj4(L, w0, wn, fta=None, ftb=None, k0=0):
                # wn (<=4) projection windows into one PSUM bank, one wide
                # scale instr, per-window tin writes
                ppj = pjpool.tile([128, wn * 128], dt.float32, name=f"pj{L}",
                                  tag="pj", padded_shape=[128, 512])
                for k in range(wn):
                    w = w0 + k
                    osl = ppj[:, k * 128:(k + 1) * 128]
                    if L == 0:
                        nc.tensor.matmul(osl,
                                         lhsT=fta[:, (k0 + k) * 128:(k0 + k + 1) * 128],
                                         rhs=w1a_s[:], start=True, stop=False)
                        nc.tensor.matmul(osl,
                                         lhsT=ftb[:, (k0 + k) * 128:(k0 + k + 1) * 128],
                                         rhs=w1b_s[:], start=False, stop=True)
                    else:
                        rhs = w2_s if L == 1 else w3_s
                        nc.tensor.matmul(osl,
                                         lhsT=h_s[:, w * 128:(w + 1) * 128],
                                         rhs=rhs[:], start=True, stop=True)
                scal = ns_s if L == 0 else nsd_s
                ssl = scal[:, w0:w0 + wn]
                sbc = AP(ssl.tensor, ssl.offset, list(ssl.ap) + [[0, 128]])
                pbf = wpool.tile([128, wn * 128], dt.bfloat16, name="pbf",
                                 tag="pbf", padded_shape=[128, 512])
                nc.vector.tensor_tensor(out=pbf[:, 0:wn * 128],
                                        in0=ppj[:, 0:wn * 128], in1=sbc,
                                        op=mybir.AluOpType.mult)
                for k in range(wn):
                    w = w0 + k
                    wsz = min(128, NP - w * 128)
                    nc.sync.dma_start(tins[L][w * 128:w * 128 + wsz, :],
                                      pbf[:wsz, k * 128:(k + 1) * 128])

            def ag(L, q):
                nc.gpsimd.collective_compute(
                    "AllGather", mybir.AluOpType.bypass,
                    replica_groups=[list(range(NCORE))],
                    ins=[tins[L][QCUM[q]:QCUM[q + 1], :].opt()],
                    outs=[tfulls[L][q][:].opt()],
                )

            def agg(L, s, ag_hooks=None):
                stile0 = sup_tile0[s]
                stiles = sup_tile0[s + 1] - stile0
                smms = mm_of_sup[s]
                nmm_s = len(smms)
                mm0 = sum(len(mm_of_sup[ss]) for ss in range(s))
                gs = gpool.tile([128, stiles, 128], dt.bfloat16,
                                name=f"gs{L}_{s}", tag="gs",
                                padded_shape=[128, max_sup_tiles, 128])
                for (j, off, n, cs) in calls_by_sup.get(s, []):
                    if ag_hooks and j in ag_hooks:
                        ag_hooks.pop(j)()
                    rel = off // 128 - stile0
                    nc.gpsimd.dma_gather(
                        gs[:, rel:rel + (n + 127) // 128, :],
                        tfulls[L][j][:],
                        sidx_s[:, off // 16:(off + n) // 16],
                        n, n, 128,
                        queue_num=next_queue(),
                    )
                oh = ohpool.tile([128, nmm_s * 128], dt.bfloat16,
                                 name=f"oh{L}", tag="oh",
                                 padded_shape=[128, max_sup_mm * 128])
                q = 0
                while q < nmm_s:
                    nb = min(ohchunk, nmm_s - q)
                    dsl = dcol_s[:, mm0 + q:mm0 + q + nb]
                    bcast = AP(dsl.tensor, dsl.offset,
                               list(dsl.ap) + [[0, 128]])
                    nc.vector.tensor_tensor(
                        out=oh[:, q * 128:(q + nb) * 128],
                        in0=iota_s[:, 0:nb * 128],
                        in1=bcast,
                        op=mybir.AluOpType.is_equal)
                    q += nb
                # per-window PSUM accumulation over this super's matmul list
                aps_of_w = {}
                done_of_w = {}
                for mi, (w, j, t) in enumerate(smms):
                    if w not in aps_of_w:
                        aps_of_w[w] = ppool.tile([128, 128], dt.float32,
                                                 name=f"ap{L}", tag="pp")
                        done_of_w[w] = 0
                    aps = aps_of_w[w]
                    k = done_of_w[w]
                    ohsl = oh[:, mi * 128:(mi + 1) * 128]
                    first, last = k == 0, k == int(nmm_w[w]) - 1
                    if L < 2:
                        nc.tensor.matmul(aps[:], lhsT=gs[:, t - stile0, :],
                                         rhs=ohsl, start=first, stop=last)
                    else:
                        nc.tensor.matmul(aps[:, 0:D_OUT], lhsT=ohsl,
                                         rhs=gs[:, t - stile0, 0:D_OUT],
                                         start=first, stop=last)
                    done_of_w[w] = k + 1
                    if not last:
                        continue
                    if L < 2:
                        nc.scalar.activation(
                            h_s[:, w * 128:(w + 1) * 128], aps[:],
                            mybir.ActivationFunctionType.Relu)
                    else:
                        wsz = min(128, NP - w * 128)
                        ob = wpool.tile([128, D_OUT], dt.float32, name="ob",
                                        tag="ob")
                        nc.vector.tensor_scalar(
                            out=ob[:], in0=aps[:, 0:D_OUT],
                            scalar1=nd_s[:, w:w + 1], scalar2=None,
                            op0=mybir.AluOpType.mult)
                        nc.sync.dma_start(
                            out_t.ap()[w * 128:w * 128 + wsz, :], ob[:wsz, :])

            # ---- L0 projection (staged featT slices) + quarter-AGs ----
            agq = 0
            for w0 in range(0, NWIN, FTW):
                wn = min(FTW, NWIN - w0)
                fta = ftpool.tile([128, wn * 128], dt.bfloat16, name="fta",
                                  tag="fta", padded_shape=[128, FTW * 128])
                ftb = ftpool.tile([128, wn * 128], dt.bfloat16, name="ftb",
                                  tag="ftb", padded_shape=[128, FTW * 128])
                nc.sync.dma_start(
                    fta[:], featT_t.ap()[0, :, w0 * 128:(w0 + wn) * 128])
                nc.sync.dma_start(
                    ftb[:], featT_t.ap()[1, :, w0 * 128:(w0 + wn) * 128])
                for g0 in range(0, wn, 4):
                    gn = min(4, wn - g0)
                    proj4(0, w0 + g0, gn, fta, ftb, g0)
                    while agq < 1 and w0 + g0 + gn - 1 >= AGWIN[agq]:
                        ag(0, agq)
                        agq += 1

            # ---- pipelined layers ----
            agsup = [min(nsup - 1, (AGWIN[q] // SUPW) + 2) for q in range(NCHUNK)]
            for L in range(3):
                hooks = ({q: (lambda qq=q: ag(0, qq)) for q in (1, 2, 3)}
                         if L == 0 else None)
                nagq = 0
                for s in range(nsup):
                    agg(L, s, ag_hooks=hooks if s == 0 else None)
                    if L < 2:
                        proj4(L + 1, sup_ws[s][0], len(sup_ws[s]))
                        while nagq < NCHUNK and s >= agsup[nagq]:
                            ag(L + 1, nagq)
                            nagq += 1

    nc.compile()
    return nc


def _in_map(consts, percore, c):
    return {
        "featT": percore["featT"][c],
        "w1": consts["w1"], "w2": consts["w2"], "w3p": consts["w3p"],
        "nsp": percore["nsp"][c], "nsdp": percore["nsdp"][c],
        "ndp": percore["ndp"][c],
        "sidx": percore["sidx"][c], "dcol": percore["dcol"][c],
        "iota": consts["iota"],
    }


def kernel(feat, W1, b1, W2, b2, W3, b3, src, dst):
    sched, consts, percore = _host_prep(feat, W1, b1, W2, b2, W3, b3, src, dst)
    nc = _build(sched)
    in_maps = [_in_map(consts, percore, c) for c in range(NCORE)]
    res = run_bass_kernel_spmd(nc, in_maps, core_ids=list(range(NCORE)))
    out = np.concatenate([res.results[c]["out"][:NP] for c in range(NCORE)],
                         axis=0)
    return np.ascontiguousarray(out.astype(np.float32))
